# revision 30
# baseline (speedup 1.0000x reference)
"""Trainium2 Bass kernel for nn_Pndb_43344809951805 (scatter_memory).

Data-parallel over batch B=16 across 8 NeuronCores (2 batches/core).

Algebraic rewrites vs the reference:
  Phase 1: scores = (questions @ Wk) @ raw^T  (q.bk bias is softmax-
           invariant over s). Wi is folded in as a 65th stationary
           column, so the v-gate logit row comes free with the scores
           matmul; sigma(g) rides the U transpose and scales the attn
           rows per-partition.
  Phase 2: one [65,512] matmul group per block (stationary = woq chunk
           plus a Wu1 column) yields the read logits transposed and the
           G1 gate row. boq enters as the exp activation's
           per-partition bias.

Cross-core reduction: AllGather of the per-core partial A [64,1024]
bf16 (lower floor than AllReduce), then a local tree-sum on DVE.
aw = A.Wu2 is computed locally post-gather; the per-s G2 gate column
comes from tiny one-column PE matmuls reusing the A2 stationary.
Post-collective emits are fused scalar_tensor_tensor ops balanced
across DVE / ACT+DVE / ACT+GpSimd.
"""
import sys

sys.path.insert(0, "/opt/trn_rl_repo")

import numpy as np
import ml_dtypes

import concourse.bass as bass
import concourse.bacc as bacc
import concourse.mybir as mybir
import concourse.tile as tile
from concourse import masks
from concourse.bass_utils import run_bass_kernel_spmd

F32 = mybir.dt.float32
BF16 = mybir.dt.bfloat16
F8 = mybir.dt.float8e4
SCL = 64.0
AF = mybir.ActivationFunctionType
ALU = mybir.AluOpType
BF = ml_dtypes.bfloat16
F8NP = ml_dtypes.float8_e4m3fn
DR = mybir.MatmulPerfMode.DoubleRow

B, S, D, Q = 16, 2048, 1024, 64
NCORES = 8
BL = B // NCORES          # local batches per core
SBLK = 512                # s-block
NSB = S // SBLK           # 4 s-blocks per batch
NCH = S // 128            # 16 s-chunks per batch
NJ = D // 128             # 8 contraction chunks
CPB = SBLK // 128         # 4 chunks per s-block
QX = Q + 1                # extra fused gate column/row
QXP = 128                 # padded stationary width: dual-fp8 LDWEIGHTS
                          # requires all 4 PE column groups active, so the
                          # stationary must span 128 columns (65.. are 0)
NBLK = BL * NSB           # 8 (b, sb) blocks per core
FREEW = D + 16            # AllGather payload row width: A cols 0:D,
                          # col D = aw (A.Wu2), rest 32B-align padding

_prog_cache = {}


def _build(bi_v: float, cgate_v: float):
    nc = bacc.Bacc("TRN2", target_bir_lowering=False, debug=False,
                   enable_asserts=False, num_devices=NCORES)

    rawT_d = nc.dram_tensor("rawT", [BL, NJ, 128, S], F8,
                            kind="ExternalInput")
    rawN_d = nc.dram_tensor("rawN", [BL * NCH, 128, D], F8,
                            kind="ExternalInput")
    pdT_d = nc.dram_tensor("pdT", [BL, NJ, 128, S], F8,
                           kind="ExternalInput")
    pdN_d = nc.dram_tensor("pdN", [BL * NCH, 128, D], BF16,
                           kind="ExternalInput")
    qkx_d = nc.dram_tensor("qkx", [NJ, 128, QXP], F8, kind="ExternalInput")
    wox_d = nc.dram_tensor("wox", [NJ, 128, QXP], F8, kind="ExternalInput")
    boq_d = nc.dram_tensor("boq", [Q, 1], F32, kind="ExternalInput")
    wu2B_d = nc.dram_tensor("wu2B", [Q, D], BF16, kind="ExternalInput")
    out_d = nc.dram_tensor("out", [BL * NCH, 128, D], BF16,
                           kind="ExternalOutput")

    with tile.TileContext(nc) as tc:
        with (
            tc.tile_pool(name="const", bufs=1) as cp,
            tc.tile_pool(name="dram", bufs=1, space="DRAM") as dram,
        ):
            # warm-up collective, dependency-free: the CC stream's
            # ~90us cold start (8-core barrier + first-collective setup)
            # runs during phase 1; contents are garbage and never read.
            # Tiny payload: a size-matched RDH warm-up measured ~45us
            # slower in the barrier+first-op chain than this Mesh one.
            ar_w = dram.tile([1, 16], BF16)
            ar_wo = dram.tile([NCORES, 16], BF16)
            nc.gpsimd.collective_compute(
                "AllGather", ALU.bypass,
                replica_groups=[list(range(NCORES))],
                ins=[ar_w.opt()], outs=[ar_wo.opt()],
            )
            ident = cp.tile([128, 128], BF16, tag="ident")
            masks.make_identity(nc, ident[:])
            nbiB = cp.tile([128, 1], F32, tag="nbiB")
            nc.vector.memset(nbiB[:], -bi_v)
            ncgB = cp.tile([128, 1], F32, tag="ncgB")
            nc.vector.memset(ncgB[:], -cgate_v)

            qkx = cp.tile([128, NJ * QXP], F8, tag="qkx")
            wox = cp.tile([128, NJ * QXP], F8, tag="wox")
            boqc = cp.tile([Q, 1], F32, tag="boqc")
            wu2B = cp.tile([Q, D], BF16, tag="wu2B")

            A_acc = cp.tile([Q, D], F32, tag="A_acc")
            # AllGather payload: [A | aw | pad] bf16; out = 8 stacked
            ag_in = dram.tile([Q, FREEW], BF16)
            ag_out = dram.tile([NCORES * Q, FREEW], BF16,
                               addr_space="Shared")

            # post-gather local sum workspace. A_bf carries the full A
            # in BOTH partition halves so the two h-half A2 matmuls can
            # run concurrently as 64-row PE tiles.
            agbuf = cp.tile([128, 4 * FREEW], BF16, tag="agbuf")
            foldlo = cp.tile([Q, FREEW], BF16, tag="foldlo")
            A_bf = cp.tile([128, FREEW], BF16, tag="A_bf")
            awjunk = cp.tile([Q, D], BF16, tag="awjunk")
            aw = cp.tile([Q, 1], F32, tag="aw")
            zpad = cp.tile([Q, 16], BF16, tag="zpad")
            nc.vector.memset(zpad[:], 0.0)
            nc.gpsimd.dma_start(ag_in[:, D:FREEW], zpad[:, 0:16])

            # shared per-block gate stats (columns = block*4 + cc)
            Z2all = cp.tile([128, NBLK * CPB], F32, tag="Z2all")
            G1all = cp.tile([128, NBLK * CPB], F32, tag="G1all")
            rzall = cp.tile([128, NBLK * CPB], F32, tag="rzall")

            # phase-2 pdT (both batches) + first pdN blocks prefetched
            # late in phase 1
            pdt0 = cp.tile([128, NJ * S], F8, tag="pdt0")
            pdt1 = cp.tile([128, NJ * S], F8, tag="pdt1")
            pdn_pre = [cp.tile([128, CPB * D], BF16, tag=f"pdnpre{k}",
                               name=f"pdnpre{k}")
                       for k in range(2)]

            def load_batchT(tile_, dram_t, b, s0=0, s1=S):
                nc.sync.dma_start(
                    tile_[:, :].rearrange("p (j s) -> p j s", j=NJ)
                    [:, :, s0:s1],
                    dram_t[b].rearrange("j p s -> p j s")[:, :, s0:s1])

            def load_n(tile_, dram_t, b, sb, eng):
                c0 = b * NCH + sb * CPB
                eng.dma_start(
                    tile_[:].rearrange("p (c d) -> p c d", c=CPB),
                    dram_t[c0:c0 + CPB].rearrange("c p d -> p c d"))

            # ================= PHASE 1 =================
            with (
                tc.tile_pool(name="p1", bufs=1) as p1,
                tc.tile_pool(name="p1ps", bufs=1, space="PSUM") as p1ps,
            ):
                def load_rn(b, sb):
                    t = p1.tile([128, CPB * D], F8, tag="rn",
                                name="rn", bufs=2)
                    load_n(t, rawN_d, b, sb, nc.scalar)
                    return t

                # weights first (tiny, needed by first matmuls)
                nc.sync.dma_start(
                    qkx[:].rearrange("p (j c) -> p j c", j=NJ),
                    qkx_d.rearrange("j p c -> p j c"))
                rawt = p1.tile([128, NJ * S], F8, tag="rawt0")
                load_batchT(rawt, rawT_d, 0, 0, SBLK)
                rn_cur = load_rn(0, 0)
                load_batchT(rawt, rawT_d, 0, SBLK, S)
                nc.sync.dma_start(
                    wox[:].rearrange("p (j c) -> p j c", j=NJ),
                    wox_d.rearrange("j p c -> p j c"))
                nc.sync.dma_start(boqc[:], boq_d[:])
                nc.gpsimd.dma_start(wu2B[:], wu2B_d[:])

                for b in range(BL):
                    Zp = p1.tile([Q, NSB], F32, tag="Zp", bufs=2)
                    A_ps = p1ps.tile([128, D], F32, tag="A_ps", bufs=2)
                    for sb in range(NSB):
                        # prefetch next block's data
                        if sb + 1 < NSB:
                            rn_nxt = load_rn(b, sb + 1)
                        elif b + 1 < BL:
                            rn_nxt = load_rn(b + 1, 0)
                        else:
                            rn_nxt = None
                        if b == 0 and sb == 0:
                            rawt_nxt = p1.tile([128, NJ * S], F8,
                                               tag="rawt1")
                            load_batchT(rawt_nxt, rawT_d, 1)
                        if b == 1 and sb == 1:
                            load_batchT(pdt0, pdT_d, 0)
                            load_n(pdn_pre[0], pdN_d, 0, 0, nc.scalar)
                            load_n(pdn_pre[1], pdN_d, 0, 1, nc.scalar)

                        # scores U[0:64] = exp(qk @ raw^T);
                        # row 64 = exp(-(raw.Wi + bi)) for the v-gate
                        sc_ps = p1ps.tile([QXP, SBLK], F32, tag="sc_ps",
                                          bufs=2)
                        qkx3 = qkx[:].rearrange("p (j c) -> p j c", j=NJ)
                        rawt3 = rawt[:].rearrange("p (j s) -> p j s", j=NJ)
                        for j in range(0, NJ, 2):
                            nc.tensor.matmul(
                                sc_ps[:], qkx3[:, j:j + 2, :],
                                rawt3[:, j:j + 2, sb * SBLK:
                                      (sb + 1) * SBLK],
                                start=(j == 0), stop=(j == NJ - 2),
                                perf_mode=DR)
                        U = p1.tile([QX, SBLK], BF16, tag="U", bufs=2)
                        nc.scalar.activation(U[0:Q, :], sc_ps[0:Q, :],
                                             AF.Exp, scale=1.0 / SCL,
                                             accum_out=Zp[:, sb:sb + 1])
                        nc.scalar.activation(U[Q:QX, :], sc_ps[Q:QX, :],
                                             AF.Exp, scale=-1.0 / SCL,
                                             bias=nbiB[0:1, :])
                        # transpose U chunks; fold g in on the way out
                        utp = None
                        rn3 = rn_cur[:].rearrange("p (c d) -> p c d",
                                                  c=CPB)
                        for cc in range(CPB):
                            ut_ps = p1ps.tile([128, QX], BF16, tag="ut_ps",
                                              bufs=2)
                            nc.tensor.transpose(
                                ut_ps[:], U[:, cc * 128:(cc + 1) * 128],
                                ident[:QX, :QX])
                            gcol = p1.tile([128, 1], F32, tag="gcol",
                                           bufs=4)
                            nc.vector.tensor_scalar_add(
                                gcol[:], ut_ps[:, Q:QX], 1.0)
                            nc.vector.reciprocal(gcol[:], gcol[:])
                            if cc % 2 == 0:
                                utp = p1.tile([128, 2 * 128], F8, tag="utp",
                                              name="utp", bufs=4)
                            nc.vector.tensor_scalar_mul(
                                utp[:, (cc % 2) * 128:
                                    (cc % 2) * 128 + Q],
                                ut_ps[:, 0:Q], gcol[:])
                            if cc % 2 == 1:
                                pr = sb * 2 + cc // 2
                                utp3 = utp[:].rearrange(
                                    "p (k m) -> p k m", k=2)
                                for h in range(2):
                                    nc.tensor.matmul(
                                        A_ps[:128, h * 512:(h + 1) * 512],
                                        utp3[:],
                                        rn3[:, cc - 1:cc + 1,
                                            h * 512:(h + 1) * 512],
                                        start=(pr == 0), stop=(pr == 7),
                                        skip_group_check=True,
                                        perf_mode=DR)
                        rn_cur = rn_nxt

                    # A_acc += A_ps / (16 * Z)
                    Z1 = p1.tile([Q, 1], F32, tag="Z1", bufs=2)
                    nc.vector.tensor_reduce(Z1[:], Zp[:], mybir.AxisListType.X,
                                            ALU.add)
                    sA = p1.tile([Q, 1], F32, tag="sA", bufs=2)
                    nc.vector.reciprocal(sA[:], Z1[:])
                    nc.vector.tensor_scalar_mul(sA[:], sA[:], 1.0 / B)
                    if b == 0:
                        nc.vector.tensor_scalar_mul(A_acc[:], A_ps[0:Q, :],
                                                    sA[:])
                        rawt = rawt_nxt
                    else:
                        nc.vector.scalar_tensor_tensor(
                            A_acc[:], A_ps[0:Q, :], sA[:], A_acc[:],
                            ALU.mult, ALU.add)

                # aw_partial = A_acc . Wu2 rides the gather as column D
                nc.vector.scalar_tensor_tensor(
                    awjunk[:], A_acc[:], 1.0, wu2B[:],
                    ALU.mult, ALU.mult, accum_out=aw[:])
                nc.gpsimd.dma_start(ag_in[:, 0:D], A_acc[:])
                nc.gpsimd.dma_start(ag_in[:, D:D + 1], aw[:])
                # pdt1 lands late on purpose: partA blocks 4-7 then run
                # inside the AllGather window, keeping the PE warm
                load_batchT(pdt1, pdT_d, 1)

            # ---- AllGather of partial A (lower floor than AllReduce;
            # the 8-way sum happens locally on DVE below) ----
            nc.gpsimd.collective_compute(
                "AllGather", ALU.bypass,
                replica_groups=[list(range(NCORES))],
                ins=[ag_in.opt()], outs=[ag_out.opt()],
            )

            # ================= PHASE 2 =================
            with (
                tc.tile_pool(name="p2", bufs=1) as p2,
                tc.tile_pool(name="p2ps", bufs=1, space="PSUM") as p2ps,
            ):
                # ---- partA: s2T + exp + transpose (Z2/G1 stats).
                # Emitted BEFORE the gather readback so the pdn loads sit
                # ahead of the collective-gated DMAs in the engine queues.
                pending = []
                pdt_cur = pdt0
                pdn_queue = []
                for idx in range(NBLK):
                    b, sb = idx // NSB, idx % NSB
                    if b == 1 and sb == 0:
                        pdt_cur = pdt1
                    if idx + 2 < NBLK:
                        nb, nsb2 = (idx + 2) // NSB, (idx + 2) % NSB
                        pdn_n = p2.tile([128, CPB * D], BF16, tag="pdn",
                                        name="pdn", bufs=6)
                        load_n(pdn_n, pdN_d, nb, nsb2, nc.sync)
                        pdn_queue.append(pdn_n)
                    pdn = pdn_pre[idx] if idx < 2 else pdn_queue.pop(0)

                    s2t_ps = p2ps.tile([QXP, SBLK], F32, tag="s2t_ps",
                                       bufs=1)
                    wox3 = wox[:].rearrange("p (j c) -> p j c", j=NJ)
                    pdt3 = pdt_cur[:].rearrange("p (j s) -> p j s", j=NJ)
                    for j in range(0, NJ, 2):
                        nc.tensor.matmul(
                            s2t_ps[:], wox3[:, j:j + 2, :],
                            pdt3[:, j:j + 2,
                                 sb * SBLK:(sb + 1) * SBLK],
                            start=(j == 0), stop=(j == NJ - 2),
                            perf_mode=DR)
                    ut2x = p2.tile([QX, SBLK], BF16, tag="ut2x", bufs=8)
                    nc.scalar.activation(ut2x[0:Q, :], s2t_ps[0:Q, :],
                                         AF.Exp, scale=1.0 / SCL,
                                         bias=boqc[:])
                    nc.scalar.activation(ut2x[Q:QX, :], s2t_ps[Q:QX, :],
                                         AF.Copy, scale=1.0 / SCL)
                    # stationary copy at partitions 64:128 for the
                    # row-tiled h1 matmuls (gpsimd ring, idle pre-gather)
                    ut2hi = p2.tile([128, SBLK], BF16, tag="ut2hi",
                                    name="ut2hi", bufs=8)
                    nc.gpsimd.dma_start(ut2hi[Q:128, :], ut2x[0:Q, :])
                    for cc in range(CPB):
                        g = idx * CPB + cc
                        u2c_ps = p2ps.tile([128, QX], BF16, tag="u2c_ps",
                                           bufs=1)
                        nc.tensor.transpose(
                            u2c_ps[:], ut2x[:, cc * 128:(cc + 1) * 128],
                            ident[:QX, :QX])
                        # Z2/G1 stats on DVE (idle pre-gather) — the ACT
                        # copy+accum chain here was partA's critical path
                        nc.vector.tensor_reduce(
                            Z2all[:, g:g + 1], u2c_ps[:, 0:Q],
                            mybir.AxisListType.X, ALU.add)
                        nc.vector.tensor_scalar_add(
                            G1all[:, g:g + 1], u2c_ps[:, Q:QX], 0.0)
                    pending.append((idx, ut2x, ut2hi, pdn))
                # 1/Z2 for every block, before the gather lands
                nc.vector.reciprocal(rzall[:], Z2all[:])

                # Everything below consumes the AllGather. Deprioritize
                # it so the scheduler keeps all partA work ahead of the
                # gather-gated ops in every engine queue (v3 interleaved
                # them and stalled the whole machine on the collective).
                ctx_lp = tc.high_priority(offset=-(1 << 20))
                ctx_lp.__enter__()

                # ---- gather readback + local 8-way sum (GpSimd: the
                # DVE queue carries partA stats that must keep flowing
                # while the collective is still in the air) ----
                # ag_out rows = 128*rr + 64*two + p (partial r = 2*rr+two)
                ag4 = ag_out[:, :].rearrange(
                    "(rr two p) c -> (two p) rr c", rr=4, two=2)
                nc.sync.dma_start(
                    agbuf[:].rearrange("p (rr c) -> p rr c", rr=4)[:, 0:2],
                    ag4[:, 0:2])
                nc.scalar.dma_start(
                    agbuf[:].rearrange("p (rr c) -> p rr c", rr=4)[:, 2:4],
                    ag4[:, 2:4])
                nc.vector.tensor_add(agbuf[:, 0:2 * FREEW],
                                     agbuf[:, 0:2 * FREEW],
                                     agbuf[:, 2 * FREEW:4 * FREEW])
                nc.vector.tensor_add(agbuf[:, 0:FREEW],
                                     agbuf[:, 0:FREEW],
                                     agbuf[:, FREEW:2 * FREEW])
                # cross-partition fold: partitions 64:128 -> 0:64
                nc.gpsimd.dma_start(foldlo[:], agbuf[Q:128, 0:FREEW])
                nc.vector.tensor_add(A_bf[0:Q, :], agbuf[0:Q, 0:FREEW],
                                     foldlo[:])
                # duplicate A into partitions 64:128 for the h1 tiles
                nc.sync.dma_start(A_bf[Q:128, :], A_bf[0:Q, :])
                aw_bf = A_bf[0:Q, D:D + 1]

                # ---- partB ----
                # G2 for ALL blocks as one PE burst, then one batched
                # gate chain: per-block SC chains serialized partB at
                # ~3us/block through cross-engine hops.
                G2ps = p2ps.tile([128, NBLK * CPB], F32, tag="G2ps")
                for (idx, ut2x, ut2hi, pdn) in pending:
                    for cc in range(CPB):
                        g = idx * CPB + cc
                        nc.tensor.matmul(
                            G2ps[:, g:g + 1],
                            ut2x[0:Q, cc * 128:(cc + 1) * 128],
                            aw_bf, start=True, stop=True,
                            skip_group_check=True,
                            tile_position=(0, 0))
                # SC = sigmoid(G1 + G2/Z2 + cg) / Z2 for all 32 chunks
                t4 = cp.tile([128, NBLK * CPB], F32, tag="t4")
                nc.vector.tensor_mul(t4[:], G2ps[:], rzall[:])
                nc.vector.tensor_add(t4[:], t4[:], G1all[:])
                e4 = cp.tile([128, NBLK * CPB], F32, tag="e4")
                nc.scalar.activation(e4[:], t4[:], AF.Exp,
                                     scale=-1.0, bias=ncgB[:])
                nc.vector.tensor_scalar_add(e4[:], e4[:], 1.0)
                nc.vector.reciprocal(e4[:], e4[:])
                SCall = cp.tile([128, NBLK * CPB], F32, tag="SCall")
                nc.vector.tensor_mul(SCall[:], e4[:], rzall[:])

                # emit route per cc slot: DVE-stt / ACT+DVE / ACT+GP,
                # ratios tuned to measured per-op costs
                ROUTE = ['A', 'C', 'B', 'A', 'C', 'B', 'A', 'C',
                         'B', 'A', 'C', 'B', 'A', 'C', 'B', 'A']
                for (idx, ut2x, ut2hi, pdn) in pending:
                    b, sb = idx // NSB, idx % NSB
                    SC = SCall[:, idx * CPB:(idx + 1) * CPB]
                    c0 = b * NCH + sb * CPB
                    oh = p2.tile([128, CPB * D], BF16, tag="oh",
                                 name="oh", bufs=2)
                    for cc in range(CPB):
                        a2_ps = p2ps.tile([128, D], F32, tag="a2_ps",
                                          bufs=2)
                        # h0 on PE rows 0:63, h1 on rows 64:127 — the
                        # two K=64 matmuls execute concurrently
                        nc.tensor.matmul(
                            a2_ps[:, 0:512],
                            ut2x[0:Q, cc * 128:(cc + 1) * 128],
                            A_bf[0:Q, 0:512],
                            start=True, stop=True,
                            skip_group_check=True,
                            tile_position=(0, 0))
                        nc.tensor.matmul(
                            a2_ps[:, 512:1024],
                            ut2hi[Q:128, cc * 128:(cc + 1) * 128],
                            A_bf[Q:128, 512:1024],
                            start=True, stop=True,
                            skip_group_check=True,
                            tile_position=(64, 0))
                        psl = slice(cc * D, (cc + 1) * D)
                        route = ROUTE[(idx * CPB + cc) % 16]
                        if route == 'A':
                            # two 512-wide stt ops: DVE PSUM-read runs
                            # measurably faster below the bank boundary
                            for h in range(2):
                                nc.vector.scalar_tensor_tensor(
                                    oh[:, cc * D + h * 512:
                                       cc * D + (h + 1) * 512],
                                    a2_ps[:, h * 512:(h + 1) * 512],
                                    SC[:, cc:cc + 1],
                                    pdn[:, cc * D + h * 512:
                                         cc * D + (h + 1) * 512],
                                    ALU.mult, ALU.add)
                        else:
                            tmp = p2.tile([128, D], BF16, tag="tmp",
                                          name="tmp", bufs=4)
                            nc.scalar.activation(tmp[:], a2_ps[:],
                                                 AF.Copy,
                                                 scale=SC[:, cc:cc + 1])
                            eng = nc.vector if route == 'C' else nc.gpsimd
                            eng.tensor_add(oh[:, psl], tmp[:],
                                           pdn[:, psl])
                    deng = nc.sync if idx % 2 == 0 else nc.scalar
                    deng.dma_start(
                        out_d[c0:c0 + CPB].rearrange("c p d -> p c d"),
                        oh[:].rearrange("p (c d) -> p c d", c=CPB))
                ctx_lp.__exit__(None, None, None)

    nc.compile()
    return nc


def _get_prog(bi_v, cgate_v):
    key = (round(bi_v, 9), round(cgate_v, 9))
    if key not in _prog_cache:
        _prog_cache[key] = _build(bi_v, cgate_v)
    return _prog_cache[key]


def kernel(raw, post_dec, mask, questions, Wk, bk, Wi, bi, Wo, bo,
           Wu1, bu1, Wu2, bu2, b1, _trace=False):
    raw = np.asarray(raw, dtype=np.float32)
    post_dec = np.asarray(post_dec, dtype=np.float32)
    questions = np.asarray(questions, dtype=np.float32)
    Wk = np.asarray(Wk, dtype=np.float32)
    Wo = np.asarray(Wo, dtype=np.float32)

    bi_v = float(np.asarray(bi).reshape(-1)[0])
    cgate_v = float(np.asarray(bu1).reshape(-1)[0]
                    + np.asarray(bu2).reshape(-1)[0]
                    + np.asarray(b1).reshape(-1)[0])
    nc = _get_prog(bi_v, cgate_v)

    inv_sqrt_d = np.float32(1.0 / np.sqrt(D))
    inv_sqrt_q = np.float32(1.0 / np.sqrt(Q))
    # stationaries with the fused gate column
    qkx = np.zeros((D, QXP), np.float32)
    qkx[:, 0:Q] = (questions @ Wk).T * inv_sqrt_d
    qkx[:, Q] = np.asarray(Wi, np.float32).reshape(D)
    wox = np.zeros((D, QXP), np.float32)
    wox[:, 0:Q] = (questions @ Wo).T * inv_sqrt_q
    wox[:, Q] = np.asarray(Wu1, np.float32).reshape(D)
    qkx = np.ascontiguousarray(qkx.reshape(NJ, 128, QXP) * SCL).astype(F8NP)
    wox = np.ascontiguousarray(wox.reshape(NJ, 128, QXP) * SCL).astype(F8NP)
    boq = np.ascontiguousarray(
        ((questions @ np.asarray(bo, np.float32)) * inv_sqrt_q
         ).reshape(Q, 1)).astype(np.float32)
    wu2B = np.ascontiguousarray(
        np.broadcast_to(np.asarray(Wu2, np.float32).reshape(1, D),
                        (Q, D))).astype(BF)

    in_maps = []
    for r in range(NCORES):
        bs = slice(r * BL, (r + 1) * BL)
        rawT = np.ascontiguousarray(
            raw[bs].transpose(0, 2, 1)).astype(F8NP).reshape(
            BL, NJ, 128, S)
        rawN = np.ascontiguousarray(raw[bs]).astype(F8NP).reshape(
            BL * NCH, 128, D)
        pdT = np.ascontiguousarray(
            post_dec[bs].transpose(0, 2, 1)).astype(F8NP).reshape(
            BL, NJ, 128, S)
        pdN = np.ascontiguousarray(post_dec[bs]).astype(BF).reshape(
            BL * NCH, 128, D)
        in_maps.append({
            "rawT": rawT, "rawN": rawN, "pdT": pdT, "pdN": pdN,
            "qkx": qkx, "wox": wox, "boq": boq, "wu2B": wu2B,
        })

    res = run_bass_kernel_spmd(nc, in_maps, core_ids=list(range(NCORES)),
                               trace=_trace)
    out = np.concatenate(
        [res.results[r]["out"].astype(np.float32).reshape(BL, S, D)
         for r in range(NCORES)],
        axis=0)
    if _trace:
        kernel._last_result = res
    return out


# revision 31
# speedup vs baseline: 1.0197x; 1.0197x over previous
"""Trainium2 Bass kernel for nn_Pndb_43344809951805 (scatter_memory).

Data-parallel over batch B=16 across 8 NeuronCores (2 batches/core).

Algebraic rewrites vs the reference:
  Phase 1: scores = (questions @ Wk) @ raw^T  (q.bk bias is softmax-
           invariant over s). Wi is folded in as a 65th stationary
           column, so the v-gate logit row comes free with the scores
           matmul; sigma(g) rides the U transpose and scales the attn
           rows per-partition.
  Phase 2: one [65,512] matmul group per block (stationary = woq chunk
           plus a Wu1 column) yields the read logits transposed and the
           G1 gate row. boq enters as the exp activation's
           per-partition bias.

Cross-core reduction: AllGather of the per-core partial A [64,1024]
bf16 (lower floor than AllReduce), then a local tree-sum on DVE.
aw = A.Wu2 is computed locally post-gather; the per-s G2 gate column
comes from tiny one-column PE matmuls reusing the A2 stationary.
Post-collective emits are fused scalar_tensor_tensor ops balanced
across DVE / ACT+DVE / ACT+GpSimd.
"""
import sys

sys.path.insert(0, "/opt/trn_rl_repo")

import numpy as np
import ml_dtypes

import concourse.bass as bass
import concourse.bacc as bacc
import concourse.mybir as mybir
import concourse.tile as tile
from concourse import masks
from concourse.bass_utils import run_bass_kernel_spmd

F32 = mybir.dt.float32
BF16 = mybir.dt.bfloat16
F8 = mybir.dt.float8e4
SCL = 64.0
AF = mybir.ActivationFunctionType
ALU = mybir.AluOpType
BF = ml_dtypes.bfloat16
F8NP = ml_dtypes.float8_e4m3fn
DR = mybir.MatmulPerfMode.DoubleRow

B, S, D, Q = 16, 2048, 1024, 64
NCORES = 8
BL = B // NCORES          # local batches per core
SBLK = 512                # s-block
NSB = S // SBLK           # 4 s-blocks per batch
NCH = S // 128            # 16 s-chunks per batch
NJ = D // 128             # 8 contraction chunks
CPB = SBLK // 128         # 4 chunks per s-block
QX = Q + 1                # extra fused gate column/row
QXP = 128                 # padded stationary width: dual-fp8 LDWEIGHTS
                          # requires all 4 PE column groups active, so the
                          # stationary must span 128 columns (65.. are 0)
NBLK = BL * NSB           # 8 (b, sb) blocks per core
FREEW = D + 16            # AllGather payload row width: A cols 0:D,
                          # col D = aw (A.Wu2), rest 32B-align padding

_prog_cache = {}


def _build(bi_v: float, cgate_v: float):
    nc = bacc.Bacc("TRN2", target_bir_lowering=False, debug=False,
                   enable_asserts=False, num_devices=NCORES)

    rawT_d = nc.dram_tensor("rawT", [BL, NJ, 128, S], F8,
                            kind="ExternalInput")
    rawN_d = nc.dram_tensor("rawN", [BL * NCH, 128, D], F8,
                            kind="ExternalInput")
    pdT_d = nc.dram_tensor("pdT", [BL, NJ, 128, S], F8,
                           kind="ExternalInput")
    pdN_d = nc.dram_tensor("pdN", [BL * NCH, 128, D], BF16,
                           kind="ExternalInput")
    qkx_d = nc.dram_tensor("qkx", [NJ, 128, QXP], F8, kind="ExternalInput")
    wox_d = nc.dram_tensor("wox", [NJ, 128, QXP], F8, kind="ExternalInput")
    boq_d = nc.dram_tensor("boq", [Q, 1], F32, kind="ExternalInput")
    wu2B_d = nc.dram_tensor("wu2B", [Q, D], BF16, kind="ExternalInput")
    out_d = nc.dram_tensor("out", [BL * NCH, 128, D], BF16,
                           kind="ExternalOutput")

    with tile.TileContext(nc) as tc:
        with (
            tc.tile_pool(name="const", bufs=1) as cp,
            tc.tile_pool(name="dram", bufs=1, space="DRAM") as dram,
        ):
            # warm-up collective, dependency-free: the CC stream's
            # ~90us cold start (8-core barrier + first-collective setup)
            # runs during phase 1; contents are garbage and never read.
            # Tiny payload: a size-matched RDH warm-up measured ~45us
            # slower in the barrier+first-op chain than this Mesh one.
            ar_w = dram.tile([1, 16], BF16)
            ar_wo = dram.tile([NCORES, 16], BF16)
            nc.gpsimd.collective_compute(
                "AllGather", ALU.bypass,
                replica_groups=[list(range(NCORES))],
                ins=[ar_w.opt()], outs=[ar_wo.opt()],
            )
            ident = cp.tile([128, 128], BF16, tag="ident")
            masks.make_identity(nc, ident[:])
            nbiB = cp.tile([128, 1], F32, tag="nbiB")
            nc.vector.memset(nbiB[:], -bi_v)
            ncgB = cp.tile([128, 1], F32, tag="ncgB")
            nc.vector.memset(ncgB[:], -cgate_v)

            qkx = cp.tile([128, NJ * QXP], F8, tag="qkx")
            wox = cp.tile([128, NJ * QXP], F8, tag="wox")
            boqc = cp.tile([Q, 1], F32, tag="boqc")
            wu2B = cp.tile([Q, D], BF16, tag="wu2B")

            A_acc = cp.tile([Q, D], F32, tag="A_acc")
            # AllGather payload: [A | aw | pad] bf16; out = 8 stacked
            ag_in = dram.tile([Q, FREEW], BF16)
            ag_out = dram.tile([NCORES * Q, FREEW], BF16,
                               addr_space="Shared")

            # post-gather local sum workspace. A_bf carries the full A
            # in BOTH partition halves so the two h-half A2 matmuls can
            # run concurrently as 64-row PE tiles.
            agbuf = cp.tile([128, 4 * FREEW], BF16, tag="agbuf")
            foldlo = cp.tile([Q, FREEW], BF16, tag="foldlo")
            A_bf = cp.tile([128, FREEW], BF16, tag="A_bf")
            awjunk = cp.tile([Q, D], BF16, tag="awjunk")
            aw = cp.tile([Q, 1], F32, tag="aw")
            zpad = cp.tile([Q, 16], BF16, tag="zpad")
            nc.vector.memset(zpad[:], 0.0)
            nc.gpsimd.dma_start(ag_in[:, D:FREEW], zpad[:, 0:16])

            # shared per-block gate stats (columns = block*4 + cc)
            Z2all = cp.tile([128, NBLK * CPB], F32, tag="Z2all")
            G1all = cp.tile([128, NBLK * CPB], F32, tag="G1all")
            rzall = cp.tile([128, NBLK * CPB], F32, tag="rzall")

            # phase-2 pdT (both batches) + first pdN blocks prefetched
            # late in phase 1
            pdt0 = cp.tile([128, NJ * S], F8, tag="pdt0")
            pdt1 = cp.tile([128, NJ * S], F8, tag="pdt1")
            pdn_pre = [cp.tile([128, CPB * D], BF16, tag=f"pdnpre{k}",
                               name=f"pdnpre{k}")
                       for k in range(2)]

            def load_batchT(tile_, dram_t, b, s0=0, s1=S):
                nc.sync.dma_start(
                    tile_[:, :].rearrange("p (j s) -> p j s", j=NJ)
                    [:, :, s0:s1],
                    dram_t[b].rearrange("j p s -> p j s")[:, :, s0:s1])

            def load_n(tile_, dram_t, b, sb, eng):
                c0 = b * NCH + sb * CPB
                eng.dma_start(
                    tile_[:].rearrange("p (c d) -> p c d", c=CPB),
                    dram_t[c0:c0 + CPB].rearrange("c p d -> p c d"))

            # ================= PHASE 1 =================
            with (
                tc.tile_pool(name="p1", bufs=1) as p1,
                tc.tile_pool(name="p1ps", bufs=1, space="PSUM") as p1ps,
            ):
                def load_rn(b, sb):
                    t = p1.tile([128, CPB * D], F8, tag="rn",
                                name="rn", bufs=3)
                    load_n(t, rawN_d, b, sb, nc.scalar)
                    return t

                # weights first (tiny, needed by first matmuls)
                nc.sync.dma_start(
                    qkx[:].rearrange("p (j c) -> p j c", j=NJ),
                    qkx_d.rearrange("j p c -> p j c"))
                rawt = p1.tile([128, NJ * S], F8, tag="rawt0")
                load_batchT(rawt, rawT_d, 0, 0, SBLK)
                rn_cur = load_rn(0, 0)
                load_batchT(rawt, rawT_d, 0, SBLK, S)
                nc.sync.dma_start(
                    wox[:].rearrange("p (j c) -> p j c", j=NJ),
                    wox_d.rearrange("j p c -> p j c"))
                nc.sync.dma_start(boqc[:], boq_d[:])
                nc.gpsimd.dma_start(wu2B[:], wu2B_d[:])

                for b in range(BL):
                    Zp = p1.tile([Q, NSB], F32, tag="Zp", bufs=2)
                    A_ps = p1ps.tile([128, D], F32, tag="A_ps", bufs=2)
                    for sb in range(NSB):
                        # prefetch next block's data
                        if sb + 1 < NSB:
                            rn_nxt = load_rn(b, sb + 1)
                        elif b + 1 < BL:
                            rn_nxt = load_rn(b + 1, 0)
                        else:
                            rn_nxt = None
                        if b == 0 and sb == 0:
                            rawt_nxt = p1.tile([128, NJ * S], F8,
                                               tag="rawt1")
                            load_batchT(rawt_nxt, rawT_d, 1)
                        if b == 1 and sb == 1:
                            load_batchT(pdt0, pdT_d, 0)
                            load_n(pdn_pre[0], pdN_d, 0, 0, nc.scalar)
                            load_n(pdn_pre[1], pdN_d, 0, 1, nc.scalar)

                        # scores U[0:64] = exp(qk @ raw^T);
                        # row 64 = exp(-(raw.Wi + bi)) for the v-gate
                        sc_ps = p1ps.tile([QXP, SBLK], F32, tag="sc_ps",
                                          bufs=2)
                        qkx3 = qkx[:].rearrange("p (j c) -> p j c", j=NJ)
                        rawt3 = rawt[:].rearrange("p (j s) -> p j s", j=NJ)
                        for j in range(0, NJ, 2):
                            nc.tensor.matmul(
                                sc_ps[:], qkx3[:, j:j + 2, :],
                                rawt3[:, j:j + 2, sb * SBLK:
                                      (sb + 1) * SBLK],
                                start=(j == 0), stop=(j == NJ - 2),
                                perf_mode=DR)
                        U = p1.tile([QX, SBLK], BF16, tag="U", bufs=2)
                        nc.scalar.activation(U[0:Q, :], sc_ps[0:Q, :],
                                             AF.Exp, scale=1.0 / SCL,
                                             accum_out=Zp[:, sb:sb + 1])
                        nc.scalar.activation(U[Q:QX, :], sc_ps[Q:QX, :],
                                             AF.Exp, scale=-1.0 / SCL,
                                             bias=nbiB[0:1, :])
                        # transpose U chunks; fold g in on the way out
                        utp = None
                        rn3 = rn_cur[:].rearrange("p (c d) -> p c d",
                                                  c=CPB)
                        for cc in range(CPB):
                            ut_ps = p1ps.tile([128, QX], BF16, tag="ut_ps",
                                              bufs=2)
                            nc.tensor.transpose(
                                ut_ps[:], U[:, cc * 128:(cc + 1) * 128],
                                ident[:QX, :QX])
                            gcol = p1.tile([128, 1], F32, tag="gcol",
                                           bufs=4)
                            nc.vector.tensor_scalar_add(
                                gcol[:], ut_ps[:, Q:QX], 1.0)
                            nc.vector.reciprocal(gcol[:], gcol[:])
                            if cc % 2 == 0:
                                utp = p1.tile([128, 2 * 128], F8, tag="utp",
                                              name="utp", bufs=4)
                            nc.vector.tensor_scalar_mul(
                                utp[:, (cc % 2) * 128:
                                    (cc % 2) * 128 + Q],
                                ut_ps[:, 0:Q], gcol[:])
                            if cc % 2 == 1:
                                pr = sb * 2 + cc // 2
                                utp3 = utp[:].rearrange(
                                    "p (k m) -> p k m", k=2)
                                for h in range(2):
                                    nc.tensor.matmul(
                                        A_ps[:128, h * 512:(h + 1) * 512],
                                        utp3[:],
                                        rn3[:, cc - 1:cc + 1,
                                            h * 512:(h + 1) * 512],
                                        start=(pr == 0), stop=(pr == 7),
                                        skip_group_check=True,
                                        perf_mode=DR)
                        rn_cur = rn_nxt

                    # A_acc += A_ps / (16 * Z)
                    Z1 = p1.tile([Q, 1], F32, tag="Z1", bufs=2)
                    nc.vector.tensor_reduce(Z1[:], Zp[:], mybir.AxisListType.X,
                                            ALU.add)
                    sA = p1.tile([Q, 1], F32, tag="sA", bufs=2)
                    nc.vector.reciprocal(sA[:], Z1[:])
                    nc.vector.tensor_scalar_mul(sA[:], sA[:], 1.0 / B)
                    if b == 0:
                        nc.vector.tensor_scalar_mul(A_acc[:], A_ps[0:Q, :],
                                                    sA[:])
                        rawt = rawt_nxt
                    else:
                        nc.vector.scalar_tensor_tensor(
                            A_acc[:], A_ps[0:Q, :], sA[:], A_acc[:],
                            ALU.mult, ALU.add)

                # aw_partial = A_acc . Wu2 rides the gather as column D
                nc.vector.scalar_tensor_tensor(
                    awjunk[:], A_acc[:], 1.0, wu2B[:],
                    ALU.mult, ALU.mult, accum_out=aw[:])
                nc.gpsimd.dma_start(ag_in[:, 0:D], A_acc[:])
                nc.gpsimd.dma_start(ag_in[:, D:D + 1], aw[:])
                # pdt1 lands late on purpose: partA blocks 4-7 then run
                # inside the AllGather window, keeping the PE warm
                load_batchT(pdt1, pdT_d, 1)

            # ---- AllGather of partial A (lower floor than AllReduce;
            # the 8-way sum happens locally on DVE below) ----
            nc.gpsimd.collective_compute(
                "AllGather", ALU.bypass,
                replica_groups=[list(range(NCORES))],
                ins=[ag_in.opt()], outs=[ag_out.opt()],
            )

            # ================= PHASE 2 =================
            with (
                tc.tile_pool(name="p2", bufs=1) as p2,
                tc.tile_pool(name="p2ps", bufs=1, space="PSUM") as p2ps,
            ):
                # ---- partA: s2T + exp + transpose (Z2/G1 stats).
                # Emitted BEFORE the gather readback so the pdn loads sit
                # ahead of the collective-gated DMAs in the engine queues.
                pending = []
                pdt_cur = pdt0
                pdn_queue = []
                for idx in range(NBLK):
                    b, sb = idx // NSB, idx % NSB
                    if b == 1 and sb == 0:
                        pdt_cur = pdt1
                    if idx + 2 < NBLK:
                        nb, nsb2 = (idx + 2) // NSB, (idx + 2) % NSB
                        pdn_n = p2.tile([128, CPB * D], BF16, tag="pdn",
                                        name="pdn", bufs=6)
                        load_n(pdn_n, pdN_d, nb, nsb2, nc.sync)
                        pdn_queue.append(pdn_n)
                    pdn = pdn_pre[idx] if idx < 2 else pdn_queue.pop(0)

                    s2t_ps = p2ps.tile([QXP, SBLK], F32, tag="s2t_ps",
                                       bufs=1)
                    wox3 = wox[:].rearrange("p (j c) -> p j c", j=NJ)
                    pdt3 = pdt_cur[:].rearrange("p (j s) -> p j s", j=NJ)
                    for j in range(0, NJ, 2):
                        nc.tensor.matmul(
                            s2t_ps[:], wox3[:, j:j + 2, :],
                            pdt3[:, j:j + 2,
                                 sb * SBLK:(sb + 1) * SBLK],
                            start=(j == 0), stop=(j == NJ - 2),
                            perf_mode=DR)
                    ut2x = p2.tile([QX, SBLK], BF16, tag="ut2x", bufs=8)
                    nc.scalar.activation(ut2x[0:Q, :], s2t_ps[0:Q, :],
                                         AF.Exp, scale=1.0 / SCL,
                                         bias=boqc[:])
                    nc.scalar.activation(ut2x[Q:QX, :], s2t_ps[Q:QX, :],
                                         AF.Copy, scale=1.0 / SCL)
                    # stationary copy at partitions 64:128 for the
                    # row-tiled h1 matmuls (gpsimd ring, idle pre-gather)
                    ut2hi = p2.tile([128, SBLK], BF16, tag="ut2hi",
                                    name="ut2hi", bufs=8)
                    nc.gpsimd.dma_start(ut2hi[Q:128, :], ut2x[0:Q, :])
                    for cc in range(CPB):
                        g = idx * CPB + cc
                        u2c_ps = p2ps.tile([128, QX], BF16, tag="u2c_ps",
                                           bufs=1)
                        nc.tensor.transpose(
                            u2c_ps[:], ut2x[:, cc * 128:(cc + 1) * 128],
                            ident[:QX, :QX])
                        # Z2/G1 stats on DVE (idle pre-gather) — the ACT
                        # copy+accum chain here was partA's critical path
                        nc.vector.tensor_reduce(
                            Z2all[:, g:g + 1], u2c_ps[:, 0:Q],
                            mybir.AxisListType.X, ALU.add)
                        nc.vector.tensor_scalar_add(
                            G1all[:, g:g + 1], u2c_ps[:, Q:QX], 0.0)
                    pending.append((idx, ut2x, ut2hi, pdn))
                # 1/Z2 for every block, before the gather lands
                nc.vector.reciprocal(rzall[:], Z2all[:])

                # Everything below consumes the AllGather. Deprioritize
                # it so the scheduler keeps all partA work ahead of the
                # gather-gated ops in every engine queue (v3 interleaved
                # them and stalled the whole machine on the collective).
                ctx_lp = tc.high_priority(offset=-(1 << 20))
                ctx_lp.__enter__()

                # ---- gather readback + local 8-way sum (GpSimd: the
                # DVE queue carries partA stats that must keep flowing
                # while the collective is still in the air) ----
                # ag_out rows = 128*rr + 64*two + p (partial r = 2*rr+two)
                ag4 = ag_out[:, :].rearrange(
                    "(rr two p) c -> (two p) rr c", rr=4, two=2)
                nc.sync.dma_start(
                    agbuf[:].rearrange("p (rr c) -> p rr c", rr=4)[:, 0:2],
                    ag4[:, 0:2])
                nc.scalar.dma_start(
                    agbuf[:].rearrange("p (rr c) -> p rr c", rr=4)[:, 2:4],
                    ag4[:, 2:4])
                nc.vector.tensor_add(agbuf[:, 0:2 * FREEW],
                                     agbuf[:, 0:2 * FREEW],
                                     agbuf[:, 2 * FREEW:4 * FREEW])
                nc.vector.tensor_add(agbuf[:, 0:FREEW],
                                     agbuf[:, 0:FREEW],
                                     agbuf[:, FREEW:2 * FREEW])
                # cross-partition fold: partitions 64:128 -> 0:64
                nc.gpsimd.dma_start(foldlo[:], agbuf[Q:128, 0:FREEW])
                nc.vector.tensor_add(A_bf[0:Q, :], agbuf[0:Q, 0:FREEW],
                                     foldlo[:])
                # duplicate A into partitions 64:128 for the h1 tiles
                nc.sync.dma_start(A_bf[Q:128, :], A_bf[0:Q, :])
                aw_bf = A_bf[0:Q, D:D + 1]

                # ---- partB ----
                # G2 for ALL blocks as one PE burst, then one batched
                # gate chain: per-block SC chains serialized partB at
                # ~3us/block through cross-engine hops.
                G2ps = p2ps.tile([128, NBLK * CPB], F32, tag="G2ps")
                for (idx, ut2x, ut2hi, pdn) in pending:
                    for cc in range(CPB):
                        g = idx * CPB + cc
                        nc.tensor.matmul(
                            G2ps[:, g:g + 1],
                            ut2x[0:Q, cc * 128:(cc + 1) * 128],
                            aw_bf, start=True, stop=True,
                            skip_group_check=True,
                            tile_position=(0, 0))
                # SC = sigmoid(G1 + G2/Z2 + cg) / Z2 for all 32 chunks
                t4 = cp.tile([128, NBLK * CPB], F32, tag="t4")
                nc.vector.tensor_mul(t4[:], G2ps[:], rzall[:])
                nc.vector.tensor_add(t4[:], t4[:], G1all[:])
                e4 = cp.tile([128, NBLK * CPB], F32, tag="e4")
                nc.scalar.activation(e4[:], t4[:], AF.Exp,
                                     scale=-1.0, bias=ncgB[:])
                nc.vector.tensor_scalar_add(e4[:], e4[:], 1.0)
                nc.vector.reciprocal(e4[:], e4[:])
                SCall = cp.tile([128, NBLK * CPB], F32, tag="SCall")
                nc.vector.tensor_mul(SCall[:], e4[:], rzall[:])

                # emit route per cc slot: DVE-stt / ACT+DVE / ACT+GP,
                # ratios tuned to measured per-op costs
                ROUTE = ['A', 'C', 'B', 'A', 'C', 'B', 'A', 'C',
                         'B', 'A', 'C', 'B', 'A', 'C', 'B', 'A']
                for (idx, ut2x, ut2hi, pdn) in pending:
                    b, sb = idx // NSB, idx % NSB
                    SC = SCall[:, idx * CPB:(idx + 1) * CPB]
                    c0 = b * NCH + sb * CPB
                    oh = p2.tile([128, CPB * D], BF16, tag="oh",
                                 name="oh", bufs=2)
                    for cc in range(CPB):
                        a2_ps = p2ps.tile([128, D], F32, tag="a2_ps",
                                          bufs=2)
                        # h0 on PE rows 0:63, h1 on rows 64:127 — the
                        # two K=64 matmuls execute concurrently
                        nc.tensor.matmul(
                            a2_ps[:, 0:512],
                            ut2x[0:Q, cc * 128:(cc + 1) * 128],
                            A_bf[0:Q, 0:512],
                            start=True, stop=True,
                            skip_group_check=True,
                            tile_position=(0, 0))
                        nc.tensor.matmul(
                            a2_ps[:, 512:1024],
                            ut2hi[Q:128, cc * 128:(cc + 1) * 128],
                            A_bf[Q:128, 512:1024],
                            start=True, stop=True,
                            skip_group_check=True,
                            tile_position=(64, 0))
                        psl = slice(cc * D, (cc + 1) * D)
                        route = ROUTE[(idx * CPB + cc) % 16]
                        if route == 'A':
                            # two 512-wide stt ops: DVE PSUM-read runs
                            # measurably faster below the bank boundary
                            for h in range(2):
                                nc.vector.scalar_tensor_tensor(
                                    oh[:, cc * D + h * 512:
                                       cc * D + (h + 1) * 512],
                                    a2_ps[:, h * 512:(h + 1) * 512],
                                    SC[:, cc:cc + 1],
                                    pdn[:, cc * D + h * 512:
                                         cc * D + (h + 1) * 512],
                                    ALU.mult, ALU.add)
                        else:
                            tmp = p2.tile([128, D], BF16, tag="tmp",
                                          name="tmp", bufs=4)
                            nc.scalar.activation(tmp[:], a2_ps[:],
                                                 AF.Copy,
                                                 scale=SC[:, cc:cc + 1])
                            eng = nc.vector if route == 'C' else nc.gpsimd
                            eng.tensor_add(oh[:, psl], tmp[:],
                                           pdn[:, psl])
                    deng = nc.sync if idx % 2 == 0 else nc.scalar
                    deng.dma_start(
                        out_d[c0:c0 + CPB].rearrange("c p d -> p c d"),
                        oh[:].rearrange("p (c d) -> p c d", c=CPB))
                ctx_lp.__exit__(None, None, None)

    nc.compile()
    return nc


def _get_prog(bi_v, cgate_v):
    key = (round(bi_v, 9), round(cgate_v, 9))
    if key not in _prog_cache:
        _prog_cache[key] = _build(bi_v, cgate_v)
    return _prog_cache[key]


def kernel(raw, post_dec, mask, questions, Wk, bk, Wi, bi, Wo, bo,
           Wu1, bu1, Wu2, bu2, b1, _trace=False):
    raw = np.asarray(raw, dtype=np.float32)
    post_dec = np.asarray(post_dec, dtype=np.float32)
    questions = np.asarray(questions, dtype=np.float32)
    Wk = np.asarray(Wk, dtype=np.float32)
    Wo = np.asarray(Wo, dtype=np.float32)

    bi_v = float(np.asarray(bi).reshape(-1)[0])
    cgate_v = float(np.asarray(bu1).reshape(-1)[0]
                    + np.asarray(bu2).reshape(-1)[0]
                    + np.asarray(b1).reshape(-1)[0])
    nc = _get_prog(bi_v, cgate_v)

    inv_sqrt_d = np.float32(1.0 / np.sqrt(D))
    inv_sqrt_q = np.float32(1.0 / np.sqrt(Q))
    # stationaries with the fused gate column
    qkx = np.zeros((D, QXP), np.float32)
    qkx[:, 0:Q] = (questions @ Wk).T * inv_sqrt_d
    qkx[:, Q] = np.asarray(Wi, np.float32).reshape(D)
    wox = np.zeros((D, QXP), np.float32)
    wox[:, 0:Q] = (questions @ Wo).T * inv_sqrt_q
    wox[:, Q] = np.asarray(Wu1, np.float32).reshape(D)
    qkx = np.ascontiguousarray(qkx.reshape(NJ, 128, QXP) * SCL).astype(F8NP)
    wox = np.ascontiguousarray(wox.reshape(NJ, 128, QXP) * SCL).astype(F8NP)
    boq = np.ascontiguousarray(
        ((questions @ np.asarray(bo, np.float32)) * inv_sqrt_q
         ).reshape(Q, 1)).astype(np.float32)
    wu2B = np.ascontiguousarray(
        np.broadcast_to(np.asarray(Wu2, np.float32).reshape(1, D),
                        (Q, D))).astype(BF)

    in_maps = []
    for r in range(NCORES):
        bs = slice(r * BL, (r + 1) * BL)
        rawT = np.ascontiguousarray(
            raw[bs].transpose(0, 2, 1)).astype(F8NP).reshape(
            BL, NJ, 128, S)
        rawN = np.ascontiguousarray(raw[bs]).astype(F8NP).reshape(
            BL * NCH, 128, D)
        pdT = np.ascontiguousarray(
            post_dec[bs].transpose(0, 2, 1)).astype(F8NP).reshape(
            BL, NJ, 128, S)
        pdN = np.ascontiguousarray(post_dec[bs]).astype(BF).reshape(
            BL * NCH, 128, D)
        in_maps.append({
            "rawT": rawT, "rawN": rawN, "pdT": pdT, "pdN": pdN,
            "qkx": qkx, "wox": wox, "boq": boq, "wu2B": wu2B,
        })

    res = run_bass_kernel_spmd(nc, in_maps, core_ids=list(range(NCORES)),
                               trace=_trace)
    out = np.concatenate(
        [res.results[r]["out"].astype(np.float32).reshape(BL, S, D)
         for r in range(NCORES)],
        axis=0)
    if _trace:
        kernel._last_result = res
    return out


# revision 32
# speedup vs baseline: 1.0262x; 1.0063x over previous
"""Trainium2 Bass kernel for nn_Pndb_43344809951805 (scatter_memory).

Data-parallel over batch B=16 across 8 NeuronCores (2 batches/core).

Algebraic rewrites vs the reference:
  Phase 1: scores = (questions @ Wk) @ raw^T  (q.bk bias is softmax-
           invariant over s). Wi is folded in as a 65th stationary
           column, so the v-gate logit row comes free with the scores
           matmul; sigma(g) rides the U transpose and scales the attn
           rows per-partition.
  Phase 2: one [65,512] matmul group per block (stationary = woq chunk
           plus a Wu1 column) yields the read logits transposed and the
           G1 gate row. boq enters as the exp activation's
           per-partition bias.

Cross-core reduction: AllGather of the per-core partial A [64,1024]
bf16 (lower floor than AllReduce), then a local tree-sum on DVE.
aw = A.Wu2 is computed locally post-gather; the per-s G2 gate column
comes from tiny one-column PE matmuls reusing the A2 stationary.
Post-collective emits are fused scalar_tensor_tensor ops balanced
across DVE / ACT+DVE / ACT+GpSimd.
"""
import sys

sys.path.insert(0, "/opt/trn_rl_repo")

import numpy as np
import ml_dtypes

import concourse.bass as bass
import concourse.bacc as bacc
import concourse.mybir as mybir
import concourse.tile as tile
from concourse import masks
from concourse.bass_utils import run_bass_kernel_spmd

F32 = mybir.dt.float32
BF16 = mybir.dt.bfloat16
F8 = mybir.dt.float8e4
SCL = 64.0
AF = mybir.ActivationFunctionType
ALU = mybir.AluOpType
BF = ml_dtypes.bfloat16
F8NP = ml_dtypes.float8_e4m3fn
DR = mybir.MatmulPerfMode.DoubleRow

B, S, D, Q = 16, 2048, 1024, 64
NCORES = 8
BL = B // NCORES          # local batches per core
SBLK = 512                # s-block
NSB = S // SBLK           # 4 s-blocks per batch
NCH = S // 128            # 16 s-chunks per batch
NJ = D // 128             # 8 contraction chunks
CPB = SBLK // 128         # 4 chunks per s-block
QX = Q + 1                # extra fused gate column/row
QXP = 128                 # padded stationary width: dual-fp8 LDWEIGHTS
                          # requires all 4 PE column groups active, so the
                          # stationary must span 128 columns (65.. are 0)
NBLK = BL * NSB           # 8 (b, sb) blocks per core
FREEW = D + 16            # AllGather payload row width: A cols 0:D,
                          # col D = aw (A.Wu2), rest 32B-align padding

_prog_cache = {}


def _build(bi_v: float, cgate_v: float):
    nc = bacc.Bacc("TRN2", target_bir_lowering=False, debug=False,
                   enable_asserts=False, num_devices=NCORES)

    rawT_d = nc.dram_tensor("rawT", [BL, NJ, 128, S], F8,
                            kind="ExternalInput")
    rawN_d = nc.dram_tensor("rawN", [BL * NCH, 128, D], F8,
                            kind="ExternalInput")
    pdT_d = nc.dram_tensor("pdT", [BL, NJ, 128, S], F8,
                           kind="ExternalInput")
    pdN_d = nc.dram_tensor("pdN", [BL * NCH, 128, D], BF16,
                           kind="ExternalInput")
    qkx_d = nc.dram_tensor("qkx", [NJ, 128, QXP], F8, kind="ExternalInput")
    wox_d = nc.dram_tensor("wox", [NJ, 128, QXP], F8, kind="ExternalInput")
    boq_d = nc.dram_tensor("boq", [Q, 1], F32, kind="ExternalInput")
    wu2B_d = nc.dram_tensor("wu2B", [Q, D], BF16, kind="ExternalInput")
    out_d = nc.dram_tensor("out", [BL * NCH, 128, D], BF16,
                           kind="ExternalOutput")

    with tile.TileContext(nc) as tc:
        with (
            tc.tile_pool(name="const", bufs=1) as cp,
            tc.tile_pool(name="dram", bufs=1, space="DRAM") as dram,
        ):
            # warm-up collective, dependency-free: the CC stream's
            # ~90us cold start (8-core barrier + first-collective setup)
            # runs during phase 1; contents are garbage and never read.
            # Tiny payload: a size-matched RDH warm-up measured ~45us
            # slower in the barrier+first-op chain than this Mesh one.
            ar_w = dram.tile([1, 16], BF16)
            ar_wo = dram.tile([NCORES, 16], BF16)
            nc.gpsimd.collective_compute(
                "AllGather", ALU.bypass,
                replica_groups=[list(range(NCORES))],
                ins=[ar_w.opt()], outs=[ar_wo.opt()],
            )
            ident = cp.tile([128, 128], BF16, tag="ident")
            masks.make_identity(nc, ident[:])
            nbiB = cp.tile([128, 1], F32, tag="nbiB")
            nc.vector.memset(nbiB[:], -bi_v)
            ncgB = cp.tile([128, 1], F32, tag="ncgB")
            nc.vector.memset(ncgB[:], -cgate_v)

            qkx = cp.tile([128, NJ * QXP], F8, tag="qkx")
            wox = cp.tile([128, NJ * QXP], F8, tag="wox")
            boqc = cp.tile([Q, 1], F32, tag="boqc")
            wu2B = cp.tile([Q, D], BF16, tag="wu2B")

            A_acc = cp.tile([Q, D], F32, tag="A_acc")
            # AllGather payload: [A | aw | pad] bf16; out = 8 stacked
            ag_in = dram.tile([Q, FREEW], BF16)
            ag_out = dram.tile([NCORES * Q, FREEW], BF16,
                               addr_space="Shared")

            # post-gather local sum workspace. A_bf carries the full A
            # in BOTH partition halves so the two h-half A2 matmuls can
            # run concurrently as 64-row PE tiles.
            agbuf = cp.tile([128, 4 * FREEW], BF16, tag="agbuf")
            foldlo = cp.tile([Q, FREEW], BF16, tag="foldlo")
            A_bf = cp.tile([128, FREEW], BF16, tag="A_bf")
            awjunk = cp.tile([Q, D], BF16, tag="awjunk")
            aw = cp.tile([Q, 1], F32, tag="aw")
            zpad = cp.tile([Q, 16], BF16, tag="zpad")
            nc.vector.memset(zpad[:], 0.0)
            nc.gpsimd.dma_start(ag_in[:, D:FREEW], zpad[:, 0:16])

            # shared per-block gate stats (columns = block*4 + cc)
            Z2all = cp.tile([128, NBLK * CPB], F32, tag="Z2all")
            G1all = cp.tile([128, NBLK * CPB], F32, tag="G1all")
            rzall = cp.tile([128, NBLK * CPB], F32, tag="rzall")

            # phase-2 pdT (both batches) + first pdN blocks prefetched
            # late in phase 1
            pdt0 = cp.tile([128, NJ * S], F8, tag="pdt0")
            pdt1 = cp.tile([128, NJ * S], F8, tag="pdt1")
            pdn_pre = [cp.tile([128, CPB * D], BF16, tag=f"pdnpre{k}",
                               name=f"pdnpre{k}")
                       for k in range(2)]

            def load_batchT(tile_, dram_t, b, s0=0, s1=S):
                nc.sync.dma_start(
                    tile_[:, :].rearrange("p (j s) -> p j s", j=NJ)
                    [:, :, s0:s1],
                    dram_t[b].rearrange("j p s -> p j s")[:, :, s0:s1])

            def load_n(tile_, dram_t, b, sb, eng):
                c0 = b * NCH + sb * CPB
                eng.dma_start(
                    tile_[:].rearrange("p (c d) -> p c d", c=CPB),
                    dram_t[c0:c0 + CPB].rearrange("c p d -> p c d"))

            # ================= PHASE 1 =================
            with (
                tc.tile_pool(name="p1", bufs=1) as p1,
                tc.tile_pool(name="p1ps", bufs=1, space="PSUM") as p1ps,
            ):
                def load_rn(b, sb):
                    t = p1.tile([128, CPB * D], F8, tag="rn",
                                name="rn", bufs=3)
                    load_n(t, rawN_d, b, sb, nc.scalar)
                    return t

                # weights first (tiny, needed by first matmuls)
                nc.sync.dma_start(
                    qkx[:].rearrange("p (j c) -> p j c", j=NJ),
                    qkx_d.rearrange("j p c -> p j c"))
                rawt = p1.tile([128, NJ * S], F8, tag="rawt0")
                load_batchT(rawt, rawT_d, 0, 0, SBLK)
                rn_cur = load_rn(0, 0)
                load_batchT(rawt, rawT_d, 0, SBLK, S)
                nc.sync.dma_start(
                    wox[:].rearrange("p (j c) -> p j c", j=NJ),
                    wox_d.rearrange("j p c -> p j c"))
                nc.sync.dma_start(boqc[:], boq_d[:])
                nc.gpsimd.dma_start(wu2B[:], wu2B_d[:])

                for b in range(BL):
                    Zp = p1.tile([Q, NSB], F32, tag="Zp", bufs=2)
                    A_ps = p1ps.tile([128, D], F32, tag="A_ps", bufs=2)
                    for sb in range(NSB):
                        # prefetch next block's data
                        if sb + 1 < NSB:
                            rn_nxt = load_rn(b, sb + 1)
                        elif b + 1 < BL:
                            rn_nxt = load_rn(b + 1, 0)
                        else:
                            rn_nxt = None
                        if b == 0 and sb == 0:
                            rawt_nxt = p1.tile([128, NJ * S], F8,
                                               tag="rawt1")
                            load_batchT(rawt_nxt, rawT_d, 1)
                        if b == 1 and sb == 1:
                            load_batchT(pdt0, pdT_d, 0)
                            load_n(pdn_pre[0], pdN_d, 0, 0, nc.scalar)
                            load_n(pdn_pre[1], pdN_d, 0, 1, nc.scalar)

                        # scores U[0:64] = exp(qk @ raw^T);
                        # row 64 = exp(-(raw.Wi + bi)) for the v-gate
                        sc_ps = p1ps.tile([QXP, SBLK], F32, tag="sc_ps",
                                          bufs=2)
                        qkx3 = qkx[:].rearrange("p (j c) -> p j c", j=NJ)
                        rawt3 = rawt[:].rearrange("p (j s) -> p j s", j=NJ)
                        for j in range(0, NJ, 2):
                            nc.tensor.matmul(
                                sc_ps[:], qkx3[:, j:j + 2, :],
                                rawt3[:, j:j + 2, sb * SBLK:
                                      (sb + 1) * SBLK],
                                start=(j == 0), stop=(j == NJ - 2),
                                perf_mode=DR)
                        U = p1.tile([QX, SBLK], BF16, tag="U", bufs=2)
                        nc.scalar.activation(U[0:Q, :], sc_ps[0:Q, :],
                                             AF.Exp, scale=1.0 / SCL,
                                             accum_out=Zp[:, sb:sb + 1])
                        nc.scalar.activation(U[Q:QX, :], sc_ps[Q:QX, :],
                                             AF.Exp, scale=-1.0 / SCL,
                                             bias=nbiB[0:1, :])
                        # transpose U chunks; fold g in on the way out
                        utp = None
                        rn3 = rn_cur[:].rearrange("p (c d) -> p c d",
                                                  c=CPB)
                        for cc in range(CPB):
                            ut_ps = p1ps.tile([128, QX], BF16, tag="ut_ps",
                                              bufs=2)
                            nc.tensor.transpose(
                                ut_ps[:], U[:, cc * 128:(cc + 1) * 128],
                                ident[:QX, :QX])
                            gcol = p1.tile([128, 1], F32, tag="gcol",
                                           bufs=4)
                            nc.vector.tensor_scalar_add(
                                gcol[:], ut_ps[:, Q:QX], 1.0)
                            nc.vector.reciprocal(gcol[:], gcol[:])
                            if cc % 2 == 0:
                                utp = p1.tile([128, 2 * 128], F8, tag="utp",
                                              name="utp", bufs=4)
                            nc.vector.tensor_scalar_mul(
                                utp[:, (cc % 2) * 128:
                                    (cc % 2) * 128 + Q],
                                ut_ps[:, 0:Q], gcol[:])
                            if cc % 2 == 1:
                                pr = sb * 2 + cc // 2
                                utp3 = utp[:].rearrange(
                                    "p (k m) -> p k m", k=2)
                                for h in range(2):
                                    nc.tensor.matmul(
                                        A_ps[:128, h * 512:(h + 1) * 512],
                                        utp3[:],
                                        rn3[:, cc - 1:cc + 1,
                                            h * 512:(h + 1) * 512],
                                        start=(pr == 0), stop=(pr == 7),
                                        skip_group_check=True,
                                        perf_mode=DR)
                        rn_cur = rn_nxt

                    # A_acc += A_ps / (16 * Z)
                    Z1 = p1.tile([Q, 1], F32, tag="Z1", bufs=2)
                    nc.vector.tensor_reduce(Z1[:], Zp[:], mybir.AxisListType.X,
                                            ALU.add)
                    sA = p1.tile([Q, 1], F32, tag="sA", bufs=2)
                    nc.vector.reciprocal(sA[:], Z1[:])
                    nc.vector.tensor_scalar_mul(sA[:], sA[:], 1.0 / B)
                    if b == 0:
                        nc.vector.tensor_scalar_mul(A_acc[:], A_ps[0:Q, :],
                                                    sA[:])
                        rawt = rawt_nxt
                    else:
                        nc.vector.scalar_tensor_tensor(
                            A_acc[:], A_ps[0:Q, :], sA[:], A_acc[:],
                            ALU.mult, ALU.add)

                # aw_partial = A_acc . Wu2 rides the gather as column D
                nc.vector.scalar_tensor_tensor(
                    awjunk[:], A_acc[:], 1.0, wu2B[:],
                    ALU.mult, ALU.mult, accum_out=aw[:])
                nc.gpsimd.dma_start(ag_in[:, 0:D], A_acc[:])
                nc.gpsimd.dma_start(ag_in[:, D:D + 1], aw[:])
                # pdt1 lands late on purpose: partA blocks 4-7 then run
                # inside the AllGather window, keeping the PE warm
                load_batchT(pdt1, pdT_d, 1)

            # ---- AllGather of partial A (lower floor than AllReduce;
            # the 8-way sum happens locally on DVE below) ----
            nc.gpsimd.collective_compute(
                "AllGather", ALU.bypass,
                replica_groups=[list(range(NCORES))],
                ins=[ag_in.opt()], outs=[ag_out.opt()],
            )

            # ================= PHASE 2 =================
            with (
                tc.tile_pool(name="p2", bufs=1) as p2,
                tc.tile_pool(name="p2ps", bufs=1, space="PSUM") as p2ps,
            ):
                # ---- partA: s2T + exp + transpose (Z2/G1 stats).
                # Emitted BEFORE the gather readback so the pdn loads sit
                # ahead of the collective-gated DMAs in the engine queues.
                pending = []
                pdt_cur = pdt0
                pdn_queue = []
                for idx in range(NBLK):
                    b, sb = idx // NSB, idx % NSB
                    if b == 1 and sb == 0:
                        pdt_cur = pdt1
                    if idx + 2 < NBLK:
                        nb, nsb2 = (idx + 2) // NSB, (idx + 2) % NSB
                        pdn_n = p2.tile([128, CPB * D], BF16, tag="pdn",
                                        name="pdn", bufs=6)
                        load_n(pdn_n, pdN_d, nb, nsb2, nc.sync)
                        pdn_queue.append(pdn_n)
                    pdn = pdn_pre[idx] if idx < 2 else pdn_queue.pop(0)

                    s2t_ps = p2ps.tile([QXP, SBLK], F32, tag="s2t_ps",
                                       bufs=1)
                    wox3 = wox[:].rearrange("p (j c) -> p j c", j=NJ)
                    pdt3 = pdt_cur[:].rearrange("p (j s) -> p j s", j=NJ)
                    for j in range(0, NJ, 2):
                        nc.tensor.matmul(
                            s2t_ps[:], wox3[:, j:j + 2, :],
                            pdt3[:, j:j + 2,
                                 sb * SBLK:(sb + 1) * SBLK],
                            start=(j == 0), stop=(j == NJ - 2),
                            perf_mode=DR)
                    ut2x = p2.tile([QX, SBLK], BF16, tag="ut2x", bufs=8)
                    nc.scalar.activation(ut2x[0:Q, :], s2t_ps[0:Q, :],
                                         AF.Exp, scale=1.0 / SCL,
                                         bias=boqc[:])
                    nc.scalar.activation(ut2x[Q:QX, :], s2t_ps[Q:QX, :],
                                         AF.Copy, scale=1.0 / SCL)
                    # stationary copy at partitions 64:128 for the
                    # row-tiled h1 matmuls (gpsimd ring, idle pre-gather)
                    ut2hi = p2.tile([128, SBLK], BF16, tag="ut2hi",
                                    name="ut2hi", bufs=8)
                    nc.gpsimd.dma_start(ut2hi[Q:128, :], ut2x[0:Q, :])
                    for cc in range(CPB):
                        g = idx * CPB + cc
                        u2c_ps = p2ps.tile([128, QX], BF16, tag="u2c_ps",
                                           bufs=1)
                        nc.tensor.transpose(
                            u2c_ps[:], ut2x[:, cc * 128:(cc + 1) * 128],
                            ident[:QX, :QX])
                        # Z2/G1 stats on DVE (idle pre-gather) — the ACT
                        # copy+accum chain here was partA's critical path
                        nc.vector.tensor_reduce(
                            Z2all[:, g:g + 1], u2c_ps[:, 0:Q],
                            mybir.AxisListType.X, ALU.add)
                        nc.vector.tensor_scalar_add(
                            G1all[:, g:g + 1], u2c_ps[:, Q:QX], 0.0)
                    pending.append((idx, ut2x, ut2hi, pdn))
                # 1/Z2 for every block, before the gather lands
                nc.vector.reciprocal(rzall[:], Z2all[:])

                # Everything below consumes the AllGather. Deprioritize
                # it so the scheduler keeps all partA work ahead of the
                # gather-gated ops in every engine queue (v3 interleaved
                # them and stalled the whole machine on the collective).
                ctx_lp = tc.high_priority(offset=-(1 << 20))
                ctx_lp.__enter__()

                # ---- gather readback + local 8-way sum (GpSimd: the
                # DVE queue carries partA stats that must keep flowing
                # while the collective is still in the air) ----
                # ag_out rows = 128*rr + 64*two + p (partial r = 2*rr+two)
                ag4 = ag_out[:, :].rearrange(
                    "(rr two p) c -> (two p) rr c", rr=4, two=2)
                nc.sync.dma_start(
                    agbuf[:].rearrange("p (rr c) -> p rr c", rr=4)[:, 0:2],
                    ag4[:, 0:2])
                nc.scalar.dma_start(
                    agbuf[:].rearrange("p (rr c) -> p rr c", rr=4)[:, 2:4],
                    ag4[:, 2:4])
                nc.vector.tensor_add(agbuf[:, 0:2 * FREEW],
                                     agbuf[:, 0:2 * FREEW],
                                     agbuf[:, 2 * FREEW:4 * FREEW])
                nc.vector.tensor_add(agbuf[:, 0:FREEW],
                                     agbuf[:, 0:FREEW],
                                     agbuf[:, FREEW:2 * FREEW])
                # cross-partition fold: partitions 64:128 -> 0:64
                nc.gpsimd.dma_start(foldlo[:], agbuf[Q:128, 0:FREEW])
                nc.vector.tensor_add(A_bf[0:Q, :], agbuf[0:Q, 0:FREEW],
                                     foldlo[:])
                # duplicate A into partitions 64:128 for the h1 tiles
                nc.sync.dma_start(A_bf[Q:128, :], A_bf[0:Q, :])
                aw_bf = A_bf[0:Q, D:D + 1]

                # ---- partB ----
                # G2 as two PE bursts with a batched gate chain per
                # half: per-block SC chains serialized partB at
                # ~3us/block through cross-engine hops, while one
                # all-32 chain delayed the first emits by ~5us.
                G2ps = p2ps.tile([128, NBLK * CPB], F32, tag="G2ps")
                t4 = cp.tile([128, NBLK * CPB], F32, tag="t4")
                e4 = cp.tile([128, NBLK * CPB], F32, tag="e4")
                SCall = cp.tile([128, NBLK * CPB], F32, tag="SCall")
                HB = NBLK * CPB // 2
                for half in range(2):
                    for (idx, ut2x, ut2hi, pdn) in pending[
                            half * NBLK // 2:(half + 1) * NBLK // 2]:
                        for cc in range(CPB):
                            g = idx * CPB + cc
                            nc.tensor.matmul(
                                G2ps[:, g:g + 1],
                                ut2x[0:Q, cc * 128:(cc + 1) * 128],
                                aw_bf, start=True, stop=True,
                                skip_group_check=True,
                                tile_position=(0, 0))
                    # SC = sigmoid(G1 + G2/Z2 + cg) / Z2 for this half
                    hs = slice(half * HB, (half + 1) * HB)
                    nc.vector.tensor_mul(t4[:, hs], G2ps[:, hs],
                                         rzall[:, hs])
                    nc.vector.tensor_add(t4[:, hs], t4[:, hs],
                                         G1all[:, hs])
                    nc.scalar.activation(e4[:, hs], t4[:, hs], AF.Exp,
                                         scale=-1.0, bias=ncgB[:])
                    nc.vector.tensor_scalar_add(e4[:, hs], e4[:, hs],
                                                1.0)
                    nc.vector.reciprocal(e4[:, hs], e4[:, hs])
                    nc.vector.tensor_mul(SCall[:, hs], e4[:, hs],
                                         rzall[:, hs])

                # emit route per cc slot: DVE-stt / ACT+DVE / ACT+GP,
                # ratios tuned to measured per-op costs
                ROUTE = ['A', 'C', 'B', 'A', 'C', 'B', 'A', 'C',
                         'B', 'A', 'C', 'B', 'A', 'C', 'B', 'A']
                for (idx, ut2x, ut2hi, pdn) in pending:
                    b, sb = idx // NSB, idx % NSB
                    SC = SCall[:, idx * CPB:(idx + 1) * CPB]
                    c0 = b * NCH + sb * CPB
                    oh = p2.tile([128, CPB * D], BF16, tag="oh",
                                 name="oh", bufs=2)
                    for cc in range(CPB):
                        a2_ps = p2ps.tile([128, D], F32, tag="a2_ps",
                                          bufs=2)
                        # h0 on PE rows 0:63, h1 on rows 64:127 — the
                        # two K=64 matmuls execute concurrently
                        nc.tensor.matmul(
                            a2_ps[:, 0:512],
                            ut2x[0:Q, cc * 128:(cc + 1) * 128],
                            A_bf[0:Q, 0:512],
                            start=True, stop=True,
                            skip_group_check=True,
                            tile_position=(0, 0))
                        nc.tensor.matmul(
                            a2_ps[:, 512:1024],
                            ut2hi[Q:128, cc * 128:(cc + 1) * 128],
                            A_bf[Q:128, 512:1024],
                            start=True, stop=True,
                            skip_group_check=True,
                            tile_position=(64, 0))
                        psl = slice(cc * D, (cc + 1) * D)
                        route = ROUTE[(idx * CPB + cc) % 16]
                        if route == 'A':
                            # two 512-wide stt ops: DVE PSUM-read runs
                            # measurably faster below the bank boundary
                            for h in range(2):
                                nc.vector.scalar_tensor_tensor(
                                    oh[:, cc * D + h * 512:
                                       cc * D + (h + 1) * 512],
                                    a2_ps[:, h * 512:(h + 1) * 512],
                                    SC[:, cc:cc + 1],
                                    pdn[:, cc * D + h * 512:
                                         cc * D + (h + 1) * 512],
                                    ALU.mult, ALU.add)
                        else:
                            tmp = p2.tile([128, D], BF16, tag="tmp",
                                          name="tmp", bufs=4)
                            nc.scalar.activation(tmp[:], a2_ps[:],
                                                 AF.Copy,
                                                 scale=SC[:, cc:cc + 1])
                            eng = nc.vector if route == 'C' else nc.gpsimd
                            eng.tensor_add(oh[:, psl], tmp[:],
                                           pdn[:, psl])
                    deng = nc.sync if idx % 2 == 0 else nc.scalar
                    deng.dma_start(
                        out_d[c0:c0 + CPB].rearrange("c p d -> p c d"),
                        oh[:].rearrange("p (c d) -> p c d", c=CPB))
                ctx_lp.__exit__(None, None, None)

    nc.compile()
    return nc


def _get_prog(bi_v, cgate_v):
    key = (round(bi_v, 9), round(cgate_v, 9))
    if key not in _prog_cache:
        _prog_cache[key] = _build(bi_v, cgate_v)
    return _prog_cache[key]


def kernel(raw, post_dec, mask, questions, Wk, bk, Wi, bi, Wo, bo,
           Wu1, bu1, Wu2, bu2, b1, _trace=False):
    raw = np.asarray(raw, dtype=np.float32)
    post_dec = np.asarray(post_dec, dtype=np.float32)
    questions = np.asarray(questions, dtype=np.float32)
    Wk = np.asarray(Wk, dtype=np.float32)
    Wo = np.asarray(Wo, dtype=np.float32)

    bi_v = float(np.asarray(bi).reshape(-1)[0])
    cgate_v = float(np.asarray(bu1).reshape(-1)[0]
                    + np.asarray(bu2).reshape(-1)[0]
                    + np.asarray(b1).reshape(-1)[0])
    nc = _get_prog(bi_v, cgate_v)

    inv_sqrt_d = np.float32(1.0 / np.sqrt(D))
    inv_sqrt_q = np.float32(1.0 / np.sqrt(Q))
    # stationaries with the fused gate column
    qkx = np.zeros((D, QXP), np.float32)
    qkx[:, 0:Q] = (questions @ Wk).T * inv_sqrt_d
    qkx[:, Q] = np.asarray(Wi, np.float32).reshape(D)
    wox = np.zeros((D, QXP), np.float32)
    wox[:, 0:Q] = (questions @ Wo).T * inv_sqrt_q
    wox[:, Q] = np.asarray(Wu1, np.float32).reshape(D)
    qkx = np.ascontiguousarray(qkx.reshape(NJ, 128, QXP) * SCL).astype(F8NP)
    wox = np.ascontiguousarray(wox.reshape(NJ, 128, QXP) * SCL).astype(F8NP)
    boq = np.ascontiguousarray(
        ((questions @ np.asarray(bo, np.float32)) * inv_sqrt_q
         ).reshape(Q, 1)).astype(np.float32)
    wu2B = np.ascontiguousarray(
        np.broadcast_to(np.asarray(Wu2, np.float32).reshape(1, D),
                        (Q, D))).astype(BF)

    in_maps = []
    for r in range(NCORES):
        bs = slice(r * BL, (r + 1) * BL)
        rawT = np.ascontiguousarray(
            raw[bs].transpose(0, 2, 1)).astype(F8NP).reshape(
            BL, NJ, 128, S)
        rawN = np.ascontiguousarray(raw[bs]).astype(F8NP).reshape(
            BL * NCH, 128, D)
        pdT = np.ascontiguousarray(
            post_dec[bs].transpose(0, 2, 1)).astype(F8NP).reshape(
            BL, NJ, 128, S)
        pdN = np.ascontiguousarray(post_dec[bs]).astype(BF).reshape(
            BL * NCH, 128, D)
        in_maps.append({
            "rawT": rawT, "rawN": rawN, "pdT": pdT, "pdN": pdN,
            "qkx": qkx, "wox": wox, "boq": boq, "wu2B": wu2B,
        })

    res = run_bass_kernel_spmd(nc, in_maps, core_ids=list(range(NCORES)),
                               trace=_trace)
    out = np.concatenate(
        [res.results[r]["out"].astype(np.float32).reshape(BL, S, D)
         for r in range(NCORES)],
        axis=0)
    if _trace:
        kernel._last_result = res
    return out


# revision 36
# speedup vs baseline: 1.0299x; 1.0036x over previous
"""Trainium2 Bass kernel for nn_Pndb_43344809951805 (scatter_memory).

Data-parallel over batch B=16 across 8 NeuronCores (2 batches/core).

Algebraic rewrites vs the reference:
  Phase 1: scores = (questions @ Wk) @ raw^T  (q.bk bias is softmax-
           invariant over s). Wi is folded in as a 65th stationary
           column, so the v-gate logit row comes free with the scores
           matmul; sigma(g) rides the U transpose and scales the attn
           rows per-partition.
  Phase 2: one [65,512] matmul group per block (stationary = woq chunk
           plus a Wu1 column) yields the read logits transposed and the
           G1 gate row. boq enters as the exp activation's
           per-partition bias.

Cross-core reduction: AllGather of the per-core partial A [64,1024]
bf16 (lower floor than AllReduce), then a local tree-sum on DVE.
aw = A.Wu2 is computed locally post-gather; the per-s G2 gate column
comes from tiny one-column PE matmuls reusing the A2 stationary.
Post-collective emits are fused scalar_tensor_tensor ops balanced
across DVE / ACT+DVE / ACT+GpSimd.
"""
import sys

sys.path.insert(0, "/opt/trn_rl_repo")

import numpy as np
import ml_dtypes

import concourse.bass as bass
import concourse.bacc as bacc
import concourse.mybir as mybir
import concourse.tile as tile
from concourse import masks
from concourse.bass_utils import run_bass_kernel_spmd

F32 = mybir.dt.float32
BF16 = mybir.dt.bfloat16
F8 = mybir.dt.float8e4
SCL = 64.0
AF = mybir.ActivationFunctionType
ALU = mybir.AluOpType
BF = ml_dtypes.bfloat16
F8NP = ml_dtypes.float8_e4m3fn
DR = mybir.MatmulPerfMode.DoubleRow

B, S, D, Q = 16, 2048, 1024, 64
NCORES = 8
BL = B // NCORES          # local batches per core
SBLK = 512                # s-block
NSB = S // SBLK           # 4 s-blocks per batch
NCH = S // 128            # 16 s-chunks per batch
NJ = D // 128             # 8 contraction chunks
CPB = SBLK // 128         # 4 chunks per s-block
QX = Q + 1                # extra fused gate column/row
QXP = 128                 # padded stationary width: dual-fp8 LDWEIGHTS
                          # requires all 4 PE column groups active, so the
                          # stationary must span 128 columns (65.. are 0)
NBLK = BL * NSB           # 8 (b, sb) blocks per core
FREEW = D + 16            # AllGather payload row width: A cols 0:D,
                          # col D = aw (A.Wu2), rest 32B-align padding

_prog_cache = {}


def _build(bi_v: float, cgate_v: float):
    nc = bacc.Bacc("TRN2", target_bir_lowering=False, debug=False,
                   enable_asserts=False, num_devices=NCORES)

    rawT_d = nc.dram_tensor("rawT", [BL, NJ, 128, S], F8,
                            kind="ExternalInput")
    rawN_d = nc.dram_tensor("rawN", [BL * NCH, 128, D], F8,
                            kind="ExternalInput")
    pdT_d = nc.dram_tensor("pdT", [BL, NJ, 128, S], F8,
                           kind="ExternalInput")
    pdN_d = nc.dram_tensor("pdN", [BL * NCH, 128, D], BF16,
                           kind="ExternalInput")
    qkx_d = nc.dram_tensor("qkx", [NJ, 128, QXP], F8, kind="ExternalInput")
    wox_d = nc.dram_tensor("wox", [NJ, 128, QXP], F8, kind="ExternalInput")
    boq_d = nc.dram_tensor("boq", [Q, 1], F32, kind="ExternalInput")
    wu2B_d = nc.dram_tensor("wu2B", [Q, D], BF16, kind="ExternalInput")
    out_d = nc.dram_tensor("out", [BL * NCH, 128, D], BF16,
                           kind="ExternalOutput")

    with tile.TileContext(nc) as tc:
        with (
            tc.tile_pool(name="const", bufs=1) as cp,
            tc.tile_pool(name="dram", bufs=1, space="DRAM") as dram,
        ):
            # warm-up collective, dependency-free: the CC stream's
            # ~90us cold start (8-core barrier + first-collective setup)
            # runs during phase 1; contents are garbage and never read.
            # Tiny payload: a size-matched RDH warm-up measured ~45us
            # slower in the barrier+first-op chain than this Mesh one.
            ar_w = dram.tile([1, 16], BF16)
            ar_wo = dram.tile([NCORES, 16], BF16)
            nc.gpsimd.collective_compute(
                "AllGather", ALU.bypass,
                replica_groups=[list(range(NCORES))],
                ins=[ar_w.opt()], outs=[ar_wo.opt()],
            )
            ident = cp.tile([128, 128], BF16, tag="ident")
            masks.make_identity(nc, ident[:])
            nbiB = cp.tile([128, 1], F32, tag="nbiB")
            nc.vector.memset(nbiB[:], -bi_v)
            ncgB = cp.tile([128, 1], F32, tag="ncgB")
            nc.vector.memset(ncgB[:], -cgate_v)

            qkx = cp.tile([128, NJ * QXP], F8, tag="qkx")
            wox = cp.tile([128, NJ * QXP], F8, tag="wox")
            boqc = cp.tile([Q, 1], F32, tag="boqc")
            wu2B = cp.tile([Q, D], BF16, tag="wu2B")

            A_acc = cp.tile([Q, D], F32, tag="A_acc")
            # AllGather payload: [A | aw | pad] bf16; out = 8 stacked
            ag_in = dram.tile([Q, FREEW], BF16)
            ag_out = dram.tile([NCORES * Q, FREEW], BF16,
                               addr_space="Shared")

            # post-gather local sum workspace. A_bf carries the full A
            # in BOTH partition halves so the two h-half A2 matmuls can
            # run concurrently as 64-row PE tiles.
            agbuf = cp.tile([128, 4 * FREEW], BF16, tag="agbuf")
            foldlo = cp.tile([Q, FREEW], BF16, tag="foldlo")
            A_bf = cp.tile([128, FREEW], BF16, tag="A_bf")
            awjunk = cp.tile([Q, D], BF16, tag="awjunk")
            aw = cp.tile([Q, 1], F32, tag="aw")
            zpad = cp.tile([Q, 16], BF16, tag="zpad")
            nc.vector.memset(zpad[:], 0.0)
            nc.gpsimd.dma_start(ag_in[:, D:FREEW], zpad[:, 0:16])

            # shared per-block gate stats (columns = block*4 + cc)
            Z2all = cp.tile([128, NBLK * CPB], F32, tag="Z2all")
            G1all = cp.tile([128, NBLK * CPB], F32, tag="G1all")
            rzall = cp.tile([128, NBLK * CPB], F32, tag="rzall")

            # phase-2 pdT (both batches) + first pdN blocks prefetched
            # late in phase 1
            pdt0 = cp.tile([128, NJ * S], F8, tag="pdt0")
            pdt1 = cp.tile([128, NJ * S], F8, tag="pdt1")
            pdn_pre = [cp.tile([128, CPB * D], BF16, tag=f"pdnpre{k}",
                               name=f"pdnpre{k}")
                       for k in range(2)]

            def load_batchT(tile_, dram_t, b, s0=0, s1=S):
                nc.sync.dma_start(
                    tile_[:, :].rearrange("p (j s) -> p j s", j=NJ)
                    [:, :, s0:s1],
                    dram_t[b].rearrange("j p s -> p j s")[:, :, s0:s1])

            def load_n(tile_, dram_t, b, sb, eng):
                c0 = b * NCH + sb * CPB
                eng.dma_start(
                    tile_[:].rearrange("p (c d) -> p c d", c=CPB),
                    dram_t[c0:c0 + CPB].rearrange("c p d -> p c d"))

            # ================= PHASE 1 =================
            with (
                tc.tile_pool(name="p1", bufs=1) as p1,
                tc.tile_pool(name="p1ps", bufs=1, space="PSUM") as p1ps,
            ):
                def load_rn(b, sb):
                    t = p1.tile([128, CPB * D], F8, tag="rn",
                                name="rn", bufs=3)
                    load_n(t, rawN_d, b, sb, nc.scalar)
                    return t

                # weights first (tiny, needed by first matmuls)
                nc.sync.dma_start(
                    qkx[:].rearrange("p (j c) -> p j c", j=NJ),
                    qkx_d.rearrange("j p c -> p j c"))
                rawt = p1.tile([128, NJ * S], F8, tag="rawt0")
                load_batchT(rawt, rawT_d, 0, 0, SBLK)
                rn_cur = load_rn(0, 0)
                load_batchT(rawt, rawT_d, 0, SBLK, S)
                nc.sync.dma_start(
                    wox[:].rearrange("p (j c) -> p j c", j=NJ),
                    wox_d.rearrange("j p c -> p j c"))
                nc.sync.dma_start(boqc[:], boq_d[:])
                nc.gpsimd.dma_start(wu2B[:], wu2B_d[:])

                for b in range(BL):
                    Zp = p1.tile([Q, NSB], F32, tag="Zp", bufs=2)
                    A_ps = p1ps.tile([128, D], F32, tag="A_ps", bufs=2)
                    for sb in range(NSB):
                        # prefetch next block's data
                        if sb + 1 < NSB:
                            rn_nxt = load_rn(b, sb + 1)
                        elif b + 1 < BL:
                            rn_nxt = load_rn(b + 1, 0)
                        else:
                            rn_nxt = None
                        if b == 0 and sb == 0:
                            rawt_nxt = p1.tile([128, NJ * S], F8,
                                               tag="rawt1")
                            load_batchT(rawt_nxt, rawT_d, 1)
                        if b == 1 and sb == 1:
                            load_batchT(pdt0, pdT_d, 0)
                            load_n(pdn_pre[0], pdN_d, 0, 0, nc.scalar)
                            load_n(pdn_pre[1], pdN_d, 0, 1, nc.scalar)

                        # scores U[0:64] = exp(qk @ raw^T);
                        # row 64 = exp(-(raw.Wi + bi)) for the v-gate
                        sc_ps = p1ps.tile([QXP, SBLK], F32, tag="sc_ps",
                                          bufs=2)
                        qkx3 = qkx[:].rearrange("p (j c) -> p j c", j=NJ)
                        rawt3 = rawt[:].rearrange("p (j s) -> p j s", j=NJ)
                        for j in range(0, NJ, 2):
                            nc.tensor.matmul(
                                sc_ps[:], qkx3[:, j:j + 2, :],
                                rawt3[:, j:j + 2, sb * SBLK:
                                      (sb + 1) * SBLK],
                                start=(j == 0), stop=(j == NJ - 2),
                                perf_mode=DR)
                        U = p1.tile([QX, SBLK], BF16, tag="U", bufs=2)
                        nc.scalar.activation(U[0:Q, :], sc_ps[0:Q, :],
                                             AF.Exp, scale=1.0 / SCL,
                                             accum_out=Zp[:, sb:sb + 1])
                        nc.scalar.activation(U[Q:QX, :], sc_ps[Q:QX, :],
                                             AF.Exp, scale=-1.0 / SCL,
                                             bias=nbiB[0:1, :])
                        # transpose U chunks; fold g in on the way out
                        utp = None
                        rn3 = rn_cur[:].rearrange("p (c d) -> p c d",
                                                  c=CPB)
                        for cc in range(CPB):
                            ut_ps = p1ps.tile([128, QX], BF16, tag="ut_ps",
                                              bufs=2)
                            nc.tensor.transpose(
                                ut_ps[:], U[:, cc * 128:(cc + 1) * 128],
                                ident[:QX, :QX])
                            gcol = p1.tile([128, 1], F32, tag="gcol",
                                           bufs=4)
                            nc.vector.tensor_scalar_add(
                                gcol[:], ut_ps[:, Q:QX], 1.0)
                            nc.vector.reciprocal(gcol[:], gcol[:])
                            if cc % 2 == 0:
                                utp = p1.tile([128, 2 * 128], F8, tag="utp",
                                              name="utp", bufs=4)
                            nc.vector.tensor_scalar_mul(
                                utp[:, (cc % 2) * 128:
                                    (cc % 2) * 128 + Q],
                                ut_ps[:, 0:Q], gcol[:])
                            if cc % 2 == 1:
                                pr = sb * 2 + cc // 2
                                utp3 = utp[:].rearrange(
                                    "p (k m) -> p k m", k=2)
                                for h in range(2):
                                    nc.tensor.matmul(
                                        A_ps[:128, h * 512:(h + 1) * 512],
                                        utp3[:],
                                        rn3[:, cc - 1:cc + 1,
                                            h * 512:(h + 1) * 512],
                                        start=(pr == 0), stop=(pr == 7),
                                        skip_group_check=True,
                                        perf_mode=DR)
                        rn_cur = rn_nxt

                    # A_acc += A_ps / (16 * Z)
                    Z1 = p1.tile([Q, 1], F32, tag="Z1", bufs=2)
                    nc.vector.tensor_reduce(Z1[:], Zp[:], mybir.AxisListType.X,
                                            ALU.add)
                    sA = p1.tile([Q, 1], F32, tag="sA", bufs=2)
                    nc.vector.reciprocal(sA[:], Z1[:])
                    nc.vector.tensor_scalar_mul(sA[:], sA[:], 1.0 / B)
                    if b == 0:
                        nc.vector.tensor_scalar_mul(A_acc[:], A_ps[0:Q, :],
                                                    sA[:])
                        rawt = rawt_nxt
                    else:
                        nc.vector.scalar_tensor_tensor(
                            A_acc[:], A_ps[0:Q, :], sA[:], A_acc[:],
                            ALU.mult, ALU.add)

                # aw_partial = A_acc . Wu2 rides the gather as column D
                nc.vector.scalar_tensor_tensor(
                    awjunk[:], A_acc[:], 1.0, wu2B[:],
                    ALU.mult, ALU.mult, accum_out=aw[:])
                nc.gpsimd.dma_start(ag_in[:, 0:D], A_acc[:])
                nc.gpsimd.dma_start(ag_in[:, D:D + 1], aw[:])
                # pdt1 lands late on purpose: partA blocks 4-7 then run
                # inside the AllGather window, keeping the PE warm
                load_batchT(pdt1, pdT_d, 1)

            # ---- AllGather of partial A (lower floor than AllReduce;
            # the 8-way sum happens locally on DVE below) ----
            nc.gpsimd.collective_compute(
                "AllGather", ALU.bypass,
                replica_groups=[list(range(NCORES))],
                ins=[ag_in.opt()], outs=[ag_out.opt()],
            )

            # ================= PHASE 2 =================
            with (
                tc.tile_pool(name="p2", bufs=1) as p2,
                tc.tile_pool(name="p2ps", bufs=1, space="PSUM") as p2ps,
            ):
                # ---- partA: s2T + exp + transpose (Z2/G1 stats).
                # Emitted BEFORE the gather readback so the pdn loads sit
                # ahead of the collective-gated DMAs in the engine queues.
                pending = []
                pdt_cur = pdt0
                pdn_queue = []
                for idx in range(NBLK):
                    b, sb = idx // NSB, idx % NSB
                    if b == 1 and sb == 0:
                        pdt_cur = pdt1
                    if idx + 2 < NBLK:
                        nb, nsb2 = (idx + 2) // NSB, (idx + 2) % NSB
                        pdn_n = p2.tile([128, CPB * D], BF16, tag="pdn",
                                        name="pdn", bufs=6)
                        load_n(pdn_n, pdN_d, nb, nsb2, nc.sync)
                        pdn_queue.append(pdn_n)
                    pdn = pdn_pre[idx] if idx < 2 else pdn_queue.pop(0)

                    s2t_ps = p2ps.tile([QXP, SBLK], F32, tag="s2t_ps",
                                       bufs=1)
                    wox3 = wox[:].rearrange("p (j c) -> p j c", j=NJ)
                    pdt3 = pdt_cur[:].rearrange("p (j s) -> p j s", j=NJ)
                    for j in range(0, NJ, 2):
                        nc.tensor.matmul(
                            s2t_ps[:], wox3[:, j:j + 2, :],
                            pdt3[:, j:j + 2,
                                 sb * SBLK:(sb + 1) * SBLK],
                            start=(j == 0), stop=(j == NJ - 2),
                            perf_mode=DR)
                    ut2x = p2.tile([QX, SBLK], BF16, tag="ut2x", bufs=8)
                    nc.scalar.activation(ut2x[0:Q, :], s2t_ps[0:Q, :],
                                         AF.Exp, scale=1.0 / SCL,
                                         bias=boqc[:])
                    nc.scalar.activation(ut2x[Q:QX, :], s2t_ps[Q:QX, :],
                                         AF.Copy, scale=1.0 / SCL)
                    # stationary copy at partitions 64:128 for the
                    # row-tiled h1 matmuls (gpsimd ring, idle pre-gather)
                    ut2hi = p2.tile([128, SBLK], BF16, tag="ut2hi",
                                    name="ut2hi", bufs=8)
                    nc.gpsimd.dma_start(ut2hi[Q:128, :], ut2x[0:Q, :])
                    for cc in range(CPB):
                        g = idx * CPB + cc
                        u2c_ps = p2ps.tile([128, QX], BF16, tag="u2c_ps",
                                           bufs=1)
                        nc.tensor.transpose(
                            u2c_ps[:], ut2x[:, cc * 128:(cc + 1) * 128],
                            ident[:QX, :QX])
                        # Z2/G1 stats on DVE (idle pre-gather) — the ACT
                        # copy+accum chain here was partA's critical path
                        nc.vector.tensor_reduce(
                            Z2all[:, g:g + 1], u2c_ps[:, 0:Q],
                            mybir.AxisListType.X, ALU.add)
                        nc.vector.tensor_scalar_add(
                            G1all[:, g:g + 1], u2c_ps[:, Q:QX], 0.0)
                    pending.append((idx, ut2x, ut2hi, pdn))
                # 1/Z2 for every block, before the gather lands
                nc.vector.reciprocal(rzall[:], Z2all[:])

                # Everything below consumes the AllGather. Deprioritize
                # it so the scheduler keeps all partA work ahead of the
                # gather-gated ops in every engine queue (v3 interleaved
                # them and stalled the whole machine on the collective).
                ctx_lp = tc.high_priority(offset=-(1 << 20))
                ctx_lp.__enter__()

                # ---- gather readback + local 8-way sum ----
                # ag_out rows = 128*rr + 64*two + p (partial r = 2*rr+two)
                # sync ring carries rr{0,2}, scalar ring rr{1,3}: each
                # level-1 add column range then depends on ONE ring, so
                # the adds start as soon as their ring's half lands.
                ag4 = ag_out[:, :].rearrange(
                    "(rr two p) c -> (two p) rr c", rr=4, two=2)
                agb4 = agbuf[:].rearrange("p (rr c) -> p rr c", rr=4)
                nc.sync.dma_start(agb4[:, 0:1], ag4[:, 0:1])
                nc.scalar.dma_start(agb4[:, 1:2], ag4[:, 1:2])
                nc.sync.dma_start(agb4[:, 2:3], ag4[:, 2:3])
                nc.scalar.dma_start(agb4[:, 3:4], ag4[:, 3:4])
                nc.vector.tensor_add(agbuf[:, 0:FREEW],
                                     agbuf[:, 0:FREEW],
                                     agbuf[:, 2 * FREEW:3 * FREEW])
                nc.vector.tensor_add(agbuf[:, FREEW:2 * FREEW],
                                     agbuf[:, FREEW:2 * FREEW],
                                     agbuf[:, 3 * FREEW:4 * FREEW])
                nc.vector.tensor_add(agbuf[:, 0:FREEW],
                                     agbuf[:, 0:FREEW],
                                     agbuf[:, FREEW:2 * FREEW])
                # cross-partition fold: partitions 64:128 -> 0:64,
                # split across two DMA rings
                HW_ = FREEW // 2
                nc.gpsimd.dma_start(foldlo[:, 0:HW_],
                                    agbuf[Q:128, 0:HW_])
                nc.sync.dma_start(foldlo[:, HW_:FREEW],
                                  agbuf[Q:128, HW_:FREEW])
                nc.vector.tensor_add(A_bf[0:Q, :], agbuf[0:Q, 0:FREEW],
                                     foldlo[:])
                # duplicate A into partitions 64:128 for the h1 tiles
                nc.sync.dma_start(A_bf[Q:128, 0:HW_], A_bf[0:Q, 0:HW_])
                nc.scalar.dma_start(A_bf[Q:128, HW_:FREEW],
                                    A_bf[0:Q, HW_:FREEW])
                aw_bf = A_bf[0:Q, D:D + 1]

                # ---- partB ----
                # G2 as two PE bursts with a batched gate chain per
                # half: per-block SC chains serialized partB at
                # ~3us/block through cross-engine hops, while one
                # all-32 chain delayed the first emits by ~5us.
                G2ps = p2ps.tile([128, NBLK * CPB], F32, tag="G2ps")
                t4 = cp.tile([128, NBLK * CPB], F32, tag="t4")
                e4 = cp.tile([128, NBLK * CPB], F32, tag="e4")
                SCall = cp.tile([128, NBLK * CPB], F32, tag="SCall")
                HB = NBLK * CPB // 2
                for half in range(2):
                    for (idx, ut2x, ut2hi, pdn) in pending[
                            half * NBLK // 2:(half + 1) * NBLK // 2]:
                        for cc in range(CPB):
                            g = idx * CPB + cc
                            nc.tensor.matmul(
                                G2ps[:, g:g + 1],
                                ut2x[0:Q, cc * 128:(cc + 1) * 128],
                                aw_bf, start=True, stop=True,
                                skip_group_check=True,
                                tile_position=(0, 0))
                    # SC = sigmoid(G1 + G2/Z2 + cg) / Z2 for this half
                    hs = slice(half * HB, (half + 1) * HB)
                    nc.vector.tensor_mul(t4[:, hs], G2ps[:, hs],
                                         rzall[:, hs])
                    nc.vector.tensor_add(t4[:, hs], t4[:, hs],
                                         G1all[:, hs])
                    nc.scalar.activation(e4[:, hs], t4[:, hs], AF.Exp,
                                         scale=-1.0, bias=ncgB[:])
                    nc.vector.tensor_scalar_add(e4[:, hs], e4[:, hs],
                                                1.0)
                    nc.vector.reciprocal(e4[:, hs], e4[:, hs])
                    nc.vector.tensor_mul(SCall[:, hs], e4[:, hs],
                                         rzall[:, hs])

                # emit route per cc slot: DVE-stt / ACT+DVE / ACT+GP,
                # ratios tuned to measured per-op costs
                ROUTE = ['A', 'C', 'B', 'A', 'B', 'C', 'A', 'B',
                         'C', 'A', 'B', 'B', 'A', 'C', 'B', 'A']
                for (idx, ut2x, ut2hi, pdn) in pending:
                    b, sb = idx // NSB, idx % NSB
                    SC = SCall[:, idx * CPB:(idx + 1) * CPB]
                    c0 = b * NCH + sb * CPB
                    oh = p2.tile([128, CPB * D], BF16, tag="oh",
                                 name="oh", bufs=3)
                    for cc in range(CPB):
                        a2_ps = p2ps.tile([128, D], F32, tag="a2_ps",
                                          bufs=2)
                        # h0 on PE rows 0:63, h1 on rows 64:127 — the
                        # two K=64 matmuls execute concurrently
                        nc.tensor.matmul(
                            a2_ps[:, 0:512],
                            ut2x[0:Q, cc * 128:(cc + 1) * 128],
                            A_bf[0:Q, 0:512],
                            start=True, stop=True,
                            skip_group_check=True,
                            tile_position=(0, 0))
                        nc.tensor.matmul(
                            a2_ps[:, 512:1024],
                            ut2hi[Q:128, cc * 128:(cc + 1) * 128],
                            A_bf[Q:128, 512:1024],
                            start=True, stop=True,
                            skip_group_check=True,
                            tile_position=(64, 0))
                        psl = slice(cc * D, (cc + 1) * D)
                        route = ROUTE[(idx * CPB + cc) % 16]
                        if route == 'A':
                            # two 512-wide stt ops: DVE PSUM-read runs
                            # measurably faster below the bank boundary
                            for h in range(2):
                                nc.vector.scalar_tensor_tensor(
                                    oh[:, cc * D + h * 512:
                                       cc * D + (h + 1) * 512],
                                    a2_ps[:, h * 512:(h + 1) * 512],
                                    SC[:, cc:cc + 1],
                                    pdn[:, cc * D + h * 512:
                                         cc * D + (h + 1) * 512],
                                    ALU.mult, ALU.add)
                        else:
                            tmp = p2.tile([128, D], BF16, tag="tmp",
                                          name="tmp", bufs=6)
                            nc.scalar.activation(tmp[:], a2_ps[:],
                                                 AF.Copy,
                                                 scale=SC[:, cc:cc + 1])
                            eng = nc.vector if route == 'C' else nc.gpsimd
                            eng.tensor_add(oh[:, psl], tmp[:],
                                           pdn[:, psl])
                    deng = nc.sync if idx % 2 == 0 else nc.scalar
                    deng.dma_start(
                        out_d[c0:c0 + CPB].rearrange("c p d -> p c d"),
                        oh[:].rearrange("p (c d) -> p c d", c=CPB))
                ctx_lp.__exit__(None, None, None)

    nc.compile()
    return nc


def _get_prog(bi_v, cgate_v):
    key = (round(bi_v, 9), round(cgate_v, 9))
    if key not in _prog_cache:
        _prog_cache[key] = _build(bi_v, cgate_v)
    return _prog_cache[key]


def kernel(raw, post_dec, mask, questions, Wk, bk, Wi, bi, Wo, bo,
           Wu1, bu1, Wu2, bu2, b1, _trace=False):
    raw = np.asarray(raw, dtype=np.float32)
    post_dec = np.asarray(post_dec, dtype=np.float32)
    questions = np.asarray(questions, dtype=np.float32)
    Wk = np.asarray(Wk, dtype=np.float32)
    Wo = np.asarray(Wo, dtype=np.float32)

    bi_v = float(np.asarray(bi).reshape(-1)[0])
    cgate_v = float(np.asarray(bu1).reshape(-1)[0]
                    + np.asarray(bu2).reshape(-1)[0]
                    + np.asarray(b1).reshape(-1)[0])
    nc = _get_prog(bi_v, cgate_v)

    inv_sqrt_d = np.float32(1.0 / np.sqrt(D))
    inv_sqrt_q = np.float32(1.0 / np.sqrt(Q))
    # stationaries with the fused gate column
    qkx = np.zeros((D, QXP), np.float32)
    qkx[:, 0:Q] = (questions @ Wk).T * inv_sqrt_d
    qkx[:, Q] = np.asarray(Wi, np.float32).reshape(D)
    wox = np.zeros((D, QXP), np.float32)
    wox[:, 0:Q] = (questions @ Wo).T * inv_sqrt_q
    wox[:, Q] = np.asarray(Wu1, np.float32).reshape(D)
    qkx = np.ascontiguousarray(qkx.reshape(NJ, 128, QXP) * SCL).astype(F8NP)
    wox = np.ascontiguousarray(wox.reshape(NJ, 128, QXP) * SCL).astype(F8NP)
    boq = np.ascontiguousarray(
        ((questions @ np.asarray(bo, np.float32)) * inv_sqrt_q
         ).reshape(Q, 1)).astype(np.float32)
    wu2B = np.ascontiguousarray(
        np.broadcast_to(np.asarray(Wu2, np.float32).reshape(1, D),
                        (Q, D))).astype(BF)

    in_maps = []
    for r in range(NCORES):
        bs = slice(r * BL, (r + 1) * BL)
        rawT = np.ascontiguousarray(
            raw[bs].transpose(0, 2, 1)).astype(F8NP).reshape(
            BL, NJ, 128, S)
        rawN = np.ascontiguousarray(raw[bs]).astype(F8NP).reshape(
            BL * NCH, 128, D)
        pdT = np.ascontiguousarray(
            post_dec[bs].transpose(0, 2, 1)).astype(F8NP).reshape(
            BL, NJ, 128, S)
        pdN = np.ascontiguousarray(post_dec[bs]).astype(BF).reshape(
            BL * NCH, 128, D)
        in_maps.append({
            "rawT": rawT, "rawN": rawN, "pdT": pdT, "pdN": pdN,
            "qkx": qkx, "wox": wox, "boq": boq, "wu2B": wu2B,
        })

    res = run_bass_kernel_spmd(nc, in_maps, core_ids=list(range(NCORES)),
                               trace=_trace)
    out = np.concatenate(
        [res.results[r]["out"].astype(np.float32).reshape(BL, S, D)
         for r in range(NCORES)],
        axis=0)
    if _trace:
        kernel._last_result = res
    return out
